# revision 1
# baseline (speedup 1.0000x reference)
"""BiLSTM-CRF loss kernel for 8 Trainium2 NeuronCores.

Sharding: data-parallel over batch (64 -> 8 cores x 8 rows). Each core runs
both LSTM directions for its batch shard, computes CRF emissions, runs the
CRF forward pass in the exp domain, the gold-path score, and writes the
partial sum of (forward - gold) over its 8 rows. Host sums partials / 64.

Key device-side tricks:
  - All gate nonlinearities collapse to a single tanh per step:
    sigmoid(x) = 0.5*(1+tanh(x/2)); the 0.5 input scales are folded into
    pre-scaled weight copies on the host, and doubled state (C=2c, H=2h)
    absorbs the output scales.
  - LSTM works in a transposed layout: stationary operand = Whh chunks,
    moving operand = H^T, so the pointwise math runs on all 128 partitions.
  - CRF forward runs in the exp domain: A_t = E_t * (P @ A_{t-1}) with
    P = exp(transitions)^T resident as the PE stationary operand, a ones
    column appended to P to produce running sums for periodic renorm.
"""

import sys

sys.path.insert(0, "/opt/trn_rl_repo")

import numpy as np
import ml_dtypes

import concourse.bass as bass
from concourse import bacc
import concourse.tile as tile
from concourse import mybir
from concourse.bass import IndirectOffsetOnAxis
from concourse import bass_isa
from concourse.bass_utils import run_bass_kernel_spmd
from concourse.masks import make_identity

F32 = mybir.dt.float32
BF16 = mybir.dt.bfloat16
I32 = mybir.dt.int32
ALU = mybir.AluOpType
AF = mybir.ActivationFunctionType
AX = mybir.AxisListType

B, L, E, H, C = 64, 256, 256, 256, 20
G = 4 * H
NCORES = 8
BC = B // NCORES            # batch rows per core
CH = 8                      # gate-hidden chunks of 128 (c = gate*2 + half)
NT = (L * BC) // 128        # token tiles per direction = 16
TPT = 128 // BC             # timesteps per token tile = 16
REN = 8                     # CRF renorm interval
NREN = L // REN
START, STOP = 18, 19

_CACHE = {}


def _build_module():
    nc = bacc.Bacc(None, target_bir_lowering=False, debug=False)

    # ---- DRAM I/O ----
    d_embed = nc.dram_tensor("embed_bf", [50000, E], BF16, kind="ExternalInput")
    d_idxf = nc.dram_tensor("idx_f", [128, NT], I32, kind="ExternalInput")
    d_idxb = nc.dram_tensor("idx_b", [128, NT], I32, kind="ExternalInput")
    d_wih = nc.dram_tensor("wih", [128, 2, 2, CH, 128], BF16, kind="ExternalInput")
    d_whh = nc.dram_tensor("whh", [128, 2, 2, CH, 128], BF16, kind="ExternalInput")
    d_xbias = nc.dram_tensor("xbias", [128, 2, CH], F32, kind="ExternalInput")
    d_h0 = nc.dram_tensor("h0T", [128, 2, 2, BC], BF16, kind="ExternalInput")
    d_c0 = nc.dram_tensor("c0T", [128, 2, 2, BC], F32, kind="ExternalInput")
    d_wout = nc.dram_tensor("woutT", [128, 2, 2, C], BF16, kind="ExternalInput")
    d_bout = nc.dram_tensor("bout", [C, 1], F32, kind="ExternalInput")
    d_transT = nc.dram_tensor("transT", [C, C], F32, kind="ExternalInput")
    d_tstop = nc.dram_tensor("tstop", [C, 1], F32, kind="ExternalInput")
    d_ohprev = nc.dram_tensor("ohprev", [C, BC, L], F32, kind="ExternalInput")
    d_ohcur = nc.dram_tensor("ohcur", [C, BC, L], F32, kind="ExternalInput")
    d_ohlast = nc.dram_tensor("ohlast", [C, BC], F32, kind="ExternalInput")
    d_a0 = nc.dram_tensor("a0", [C, BC], F32, kind="ExternalInput")
    d_out = nc.dram_tensor("out", [1, 1], F32, kind="ExternalOutput")

    with tile.TileContext(nc) as tc:
        with (
            tc.tile_pool(name="persist", bufs=1) as pp,
            tc.tile_pool(name="work", bufs=3) as wp,
            tc.tile_pool(name="lstm", bufs=3) as lp,
        ):
            # ---- persistent SBUF ----
            wih_sb = pp.tile([128, 2, 2, CH, 128], BF16, tag="wih")
            whh_sb = pp.tile([128, 2, 2, CH, 128], BF16, tag="whh")
            xbias_sb = pp.tile([128, 2, CH], F32, tag="xbias")
            wout_sb = pp.tile([128, 2, 2, C], BF16, tag="wout")
            bout_sb = pp.tile([C, 1], F32, tag="bout")
            transT_sb = pp.tile([C, C], F32, tag="transT")
            tstop_sb = pp.tile([C, 1], F32, tag="tstop")
            ohprev_sb = pp.tile([C, BC, L], F32, tag="ohprev")
            ohcur_sb = pp.tile([C, BC, L], F32, tag="ohcur")
            ohlast_sb = pp.tile([C, BC], F32, tag="ohlast")
            idxf_sb = pp.tile([128, NT], I32, tag="idxf")
            idxb_sb = pp.tile([128, NT], I32, tag="idxb")
            ident128 = pp.tile([128, 128], BF16, tag="id128")
            ident20 = pp.tile([C, C], F32, tag="id20")
            s_tiles = pp.tile([128, NT], F32, tag="stiles")
            s_row = pp.tile([1, L * BC], F32, tag="srow")
            # x^T staging: [E-part, k-half of E? no: k index of E chunk, tile, token]
            xTf = pp.tile([128, 2, NT, 128], BF16, tag="xTf")
            xTb = pp.tile([128, 2, NT, 128], BF16, tag="xTb")
            # xp^T: [ghid-part, t, chunk, b]
            xpT = [pp.tile([128, L, CH, BC], BF16, name=f"xpT{d}", tag=f"xpT{d}") for d in (0, 1)]
            # H history: [hid-part, k-half, t(0..L), b]   slot s+1 = state after step s
            hsT = [pp.tile([128, 2, L + 1, BC], BF16, name=f"hsT{d}", tag=f"hsT{d}") for d in (0, 1)]
            cst = [pp.tile([128, 2, BC], F32, name=f"cst{d}", tag=f"cst{d}") for d in (0, 1)]
            featsT = pp.tile([128, L, BC], F32, tag="featsT")
            eT = pp.tile([C, L, BC], F32, tag="eT")
            pplus = pp.tile([C, C], F32, tag="pplus")
            wstop = pp.tile([C, 1], F32, tag="wstop")
            sall = pp.tile([1, BC, NREN], F32, tag="sall")
            ones1 = pp.tile([1, C], F32, tag="ones1")
            ones20c = pp.tile([C, 1], F32, tag="ones20c")
            avec = pp.tile([C, 2, BC], F32, tag="avec")
            gold_sb = pp.tile([1, BC], F32, tag="gold")

            # ---- load constants ----
            nc.sync.dma_start(out=wih_sb[:], in_=d_wih[:])
            nc.sync.dma_start(out=whh_sb[:], in_=d_whh[:])
            nc.sync.dma_start(out=xbias_sb[:], in_=d_xbias[:])
            nc.sync.dma_start(out=wout_sb[:], in_=d_wout[:])
            nc.sync.dma_start(out=bout_sb[:], in_=d_bout[:])
            nc.sync.dma_start(out=transT_sb[:], in_=d_transT[:])
            nc.sync.dma_start(out=tstop_sb[:], in_=d_tstop[:])
            nc.sync.dma_start(out=ohprev_sb[:], in_=d_ohprev[:])
            nc.sync.dma_start(out=ohcur_sb[:], in_=d_ohcur[:])
            nc.sync.dma_start(out=ohlast_sb[:], in_=d_ohlast[:])
            nc.sync.dma_start(out=idxf_sb[:], in_=d_idxf[:])
            nc.sync.dma_start(out=idxb_sb[:], in_=d_idxb[:])
            for d in (0, 1):
                nc.sync.dma_start(out=hsT[d][:, :, 0, :], in_=d_h0[:, d, :, :])
                nc.sync.dma_start(out=cst[d][:], in_=d_c0[:, d, :, :])
            make_identity(nc, ident128[:])
            make_identity(nc, ident20[:])
            nc.vector.memset(ones1[:], 1.0)
            nc.vector.memset(ones20c[:], 1.0)

            # DVE staging copies so downstream DVE ops carry <=1 sync wait
            ohcur_c = pp.tile([C, BC, L], F32, tag="ohcur_c")
            nc.vector.tensor_copy(ohcur_c[:], ohcur_sb[:])
            ohlast_c = pp.tile([C, BC], F32, tag="ohlast_c")
            nc.vector.tensor_copy(ohlast_c[:], ohlast_sb[:])

            # P+ = exp(transT);  wstop = exp(T[STOP,:])
            nc.scalar.activation(pplus[:], transT_sb[:], AF.Exp)
            nc.scalar.activation(wstop[:], tstop_sb[:], AF.Exp)

            # A0 = onehot(START) in slot 0
            nc.vector.memset(avec[:], 0.0)
            nc.sync.dma_start(out=avec[:, 0, :], in_=d_a0[:])

            # ---- P1: gather + transpose + input projection ----
            ps_p1 = tc.tile_pool(name="ps_p1", bufs=2, space="PSUM")
            psA = psB = ps_p1.__enter__()
            for d in (0, 1):
                idx_sb = idxf_sb if d == 0 else idxb_sb
                xT = xTf if d == 0 else xTb
                for ti in range(NT):
                    gx = wp.tile([128, E], BF16, tag="gx")
                    nc.gpsimd.indirect_dma_start(
                        out=gx[:],
                        out_offset=None,
                        in_=d_embed[:],
                        in_offset=IndirectOffsetOnAxis(ap=idx_sb[:, ti : ti + 1], axis=0),
                    )
                    for k in (0, 1):
                        pt = psB.tile([128, 128], BF16, tag="ptr")
                        nc.tensor.transpose(
                            pt[:], gx[:, k * 128 : (k + 1) * 128], ident128[:]
                        )
                        nc.vector.tensor_copy(xT[:, k, ti, :], pt[:])
            for d in (0, 1):
                xT = xTf if d == 0 else xTb
                for c in range(CH):
                    for h in (0, 1):
                        pj = psA.tile([128, 8, 128], F32, tag="pj", bufs=2)
                        for tj in range(8):
                            for k in (0, 1):
                                nc.tensor.matmul(
                                    pj[:, tj, :],
                                    wih_sb[:, d, k, c, :],
                                    xT[:, k, h * 8 + tj, :],
                                    start=(k == 0),
                                    stop=(k == 1),
                                )
                        # xp^T[t, c, b] = pj + bias
                        nc.vector.tensor_scalar(
                            out=xpT[d][:, h * 128 : (h + 1) * 128, c, :],
                            in0=pj[:].rearrange(
                                "p tj (tl b) -> p (tj tl) b", b=BC
                            ),
                            scalar1=xbias_sb[:, d, c : c + 1],
                            scalar2=None,
                            op0=ALU.add,
                        )
            ps_p1.__exit__(None, None, None)

            # ---- P2: LSTM recurrence (both directions interleaved) ----
            ps_p2 = tc.tile_pool(name="ps_p2", bufs=4, space="PSUM")
            psB = ps_p2.__enter__()
            for s in range(L):
                for d in (0, 1):
                    pg = psB.tile([128, CH, BC], F32, tag="pg")
                    for c in range(CH):
                        for k in (0, 1):
                            nc.tensor.matmul(
                                pg[:, c, :],
                                whh_sb[:, d, k, c, :],
                                hsT[d][:, k, s, :],
                                start=(k == 0),
                                stop=(k == 1),
                            )
                    gt = lp.tile([128, CH, BC], F32, tag="gt")
                    nc.vector.scalar_tensor_tensor(
                        out=gt[:], in0=pg[:], scalar=1.0, in1=xpT[d][:, s, :, :],
                        op0=ALU.mult, op1=ALU.add,
                    )
                    th = lp.tile([128, CH, BC], F32, tag="th")
                    nc.scalar.activation(th[:], gt[:], AF.Tanh)
                    u = lp.tile([128, 2, BC], F32, tag="u")
                    nc.vector.scalar_tensor_tensor(
                        out=u[:], in0=th[:, 0:2, :], scalar=1.0, in1=th[:, 4:6, :],
                        op0=ALU.add, op1=ALU.mult,
                    )
                    v = lp.tile([128, 2, BC], F32, tag="v")
                    nc.vector.scalar_tensor_tensor(
                        out=v[:], in0=th[:, 2:4, :], scalar=1.0, in1=cst[d][:],
                        op0=ALU.add, op1=ALU.mult,
                    )
                    nc.vector.scalar_tensor_tensor(
                        out=cst[d][:], in0=v[:], scalar=0.5, in1=u[:],
                        op0=ALU.mult, op1=ALU.add,
                    )
                    tcc = lp.tile([128, 2, BC], F32, tag="tcc")
                    nc.scalar.activation(tcc[:], cst[d][:], AF.Tanh, scale=0.5)
                    nc.vector.scalar_tensor_tensor(
                        out=hsT[d][:, :, s + 1, :], in0=th[:, 6:8, :], scalar=1.0,
                        in1=tcc[:], op0=ALU.add, op1=ALU.mult,
                    )

            ps_p2.__exit__(None, None, None)

            # ---- P3: emissions feats^T = sum_d Wout_d @ H_d + bout ----
            ps_p3 = tc.tile_pool(name="ps_p3", bufs=1, space="PSUM")
            psA = psB = ps_p3.__enter__()
            pf = psA.tile([C, L * BC], F32, tag="big")
            for d in (0, 1):
                for k in (0, 1):
                    for n in range(4):
                        nc.tensor.matmul(
                            pf[:, n * 512 : (n + 1) * 512],
                            wout_sb[:, d, k, :],
                            hsT[d][:, k, 1 + n * 64 : 1 + (n + 1) * 64, :],
                            start=(d == 0 and k == 0),
                            stop=(d == 1 and k == 1),
                        )
            nc.vector.memset(featsT[:], -3.0e38)
            nc.vector.scalar_tensor_tensor(
                out=featsT[0:C].rearrange("p t b -> p (t b)"),
                in0=pf[:],
                scalar=1.0,
                in1=bout_sb[:].to_broadcast([C, L * BC]),
                op0=ALU.mult,
                op1=ALU.add,
            )

            # ---- P4: CRF prep ----
            # per-(t,b) max over tags via PE transpose + free-dim reduce
            for ti in range(NT):
                ptf = psB.tile([128, C], F32, tag="ptf", bufs=2)
                nc.tensor.transpose(
                    ptf[:], featsT[0:C, ti * TPT : (ti + 1) * TPT, :], ident20[:]
                )
                nc.vector.tensor_reduce(
                    out=s_tiles[:, ti : ti + 1], in_=ptf[:], axis=AX.X, op=ALU.max
                )

            # s_row[0, p*NT + ti] = s_tiles[p, ti]  (one DMA, one sem lane)
            nc.sync.dma_start(out=s_row[0:1, :], in_=s_tiles[:])
            # broadcast s over tag partitions (ones-column matmul); rhs view
            # reorders (tl, b, ti) -> feats order (ti, tl, b)
            sv = s_row[0:1, :].rearrange(
                "a (tl b ti) -> a ti tl b", b=BC, ti=NT
            )
            psb = psA.tile([C, L * BC], F32, tag="big")
            for n in range(4):
                nc.tensor.matmul(
                    psb[:, n * 512 : (n + 1) * 512],
                    ones1[:],
                    sv[:, n * 4 : (n + 1) * 4, :, :],
                    start=True,
                    stop=True,
                )
            fm = pp.tile([C, L * BC], F32, tag="fm")
            nc.vector.scalar_tensor_tensor(
                out=fm[:], in0=featsT[0:C].rearrange("p t b -> p (t b)"), scalar=0.0, in1=psb[:], op0=ALU.add, op1=ALU.subtract,
            )
            nc.scalar.activation(fm[:], fm[:], AF.Exp)
            nc.vector.tensor_copy(eT[:].rearrange("p t b -> p (t b)"), fm[:])

            # gold score: U = T @ ohprev ; transum = sum_t (U * ohcur)
            pu = psA.tile([C, BC * L], F32, tag="big")
            for n in range(4):
                nc.tensor.matmul(
                    pu[:, n * 512 : (n + 1) * 512],
                    transT_sb[:],
                    ohprev_sb[:].rearrange("p b t -> p (b t)")[
                        :, n * 512 : (n + 1) * 512
                    ],
                    start=True,
                    stop=True,
                )
            prod = pp.tile([C, BC, L], F32, tag="prod")
            nc.vector.scalar_tensor_tensor(
                out=prod[:].rearrange("p b t -> p (b t)"), in0=pu[:], scalar=0.0, in1=ohcur_c[:].rearrange("p b t -> p (b t)"), op0=ALU.add, op1=ALU.mult,
            )
            gsum = pp.tile([C, BC], F32, tag="gsum")
            nc.vector.tensor_reduce(out=gsum[:], in_=prod[:], axis=AX.X, op=ALU.add)
            # emissions at gold tags: featsT viewed [C, b, t] * ohcur
            prod2 = pp.tile([C, BC, L], F32, tag="prod2")
            nc.vector.scalar_tensor_tensor(
                out=prod2[:], in0=featsT[0:C].rearrange("p t b -> p b t"), scalar=0.0, in1=ohcur_c[:], op0=ALU.add, op1=ALU.mult,
            )
            gsum2 = pp.tile([C, BC], F32, tag="gsum2")
            nc.vector.tensor_reduce(out=gsum2[:], in_=prod2[:], axis=AX.X, op=ALU.add)
            nc.vector.scalar_tensor_tensor(
                out=gsum[:], in0=gsum[:], scalar=0.0, in1=gsum2[:], op0=ALU.add, op1=ALU.add,
            )
            # + T[STOP, tag_last]
            stoption = pp.tile([C, BC], F32, tag="stopterm")
            nc.vector.scalar_tensor_tensor(
                out=stoption[:], in0=ohlast_c[:], scalar=0.0, in1=tstop_sb[:].to_broadcast([C, BC]), op0=ALU.add, op1=ALU.mult,
            )
            nc.vector.scalar_tensor_tensor(
                out=gsum[:], in0=gsum[:], scalar=0.0, in1=stoption[:], op0=ALU.add, op1=ALU.add,
            )
            pgold = psB.tile([1, BC], F32, tag="pgold")
            nc.tensor.matmul(pgold[:], ones20c[:], gsum[:], start=True, stop=True)
            nc.vector.tensor_copy(gold_sb[:], pgold[:])

            ps_p3.__exit__(None, None, None)

            # ---- P5: CRF forward scan (exp domain) ----
            ps_p5 = tc.tile_pool(name="ps_p5", bufs=2, space="PSUM")
            psB = ps_p5.__enter__()
            for t in range(L):
                pa = psB.tile([C, BC], F32, tag="pa")
                nc.tensor.matmul(
                    pa[:], pplus[:], avec[:, t % 2, :], start=True, stop=True
                )
                nc.vector.scalar_tensor_tensor(
                out=avec[:, (t + 1) % 2, :], in0=pa[:], scalar=0.0, in1=eT[:, t, :], op0=ALU.add, op1=ALU.mult,
            )
                if t % REN == REN - 1:
                    rn = t // REN
                    pss = psB.tile([1, BC], F32, tag="pss")
                    nc.tensor.matmul(
                        pss[:], ones20c[:], avec[:, (t + 1) % 2, :],
                        start=True, stop=True,
                    )
                    nc.vector.tensor_copy(sall[0:1, :, rn], pss[:])
                    srec = wp.tile([1, BC], F32, tag="srec")
                    nc.vector.reciprocal(srec[:], pss[:])
                    pb = psB.tile([C, BC], F32, tag="pb")
                    nc.tensor.matmul(
                        pb[:], ones1[:], srec[:], start=True, stop=True
                    )
                    nc.vector.scalar_tensor_tensor(
                out=avec[:, (t + 1) % 2, :], in0=avec[:, (t + 1) % 2, :], scalar=0.0, in1=pb[:], op0=ALU.add, op1=ALU.mult,
            )

            # ---- P6: finalization ----
            paf = psB.tile([1, BC], F32, tag="paf", bufs=1)
            nc.tensor.matmul(
                paf[:], wstop[:], avec[:, L % 2, :], start=True, stop=True
            )
            flog = wp.tile([1, BC], F32, tag="flog")
            nc.scalar.activation(flog[:], paf[:], AF.Ln)
            slog = wp.tile([1, BC, NREN], F32, tag="slog")
            nc.scalar.activation(slog[:], sall[:], AF.Ln)
            slogsum = wp.tile([1, BC], F32, tag="slogsum")
            nc.vector.tensor_reduce(out=slogsum[:], in_=slog[:], axis=AX.X, op=ALU.add)
            # sum of per-step shifts s_t
            ssum = wp.tile([1, BC], F32, tag="ssum")
            nc.vector.tensor_reduce(
                out=ssum[:],
                in_=s_row[0:1, :].rearrange("a (tl b ti) -> a b tl ti", b=BC, ti=NT),
                axis=AX.XY,
                op=ALU.add,
            )
            # F - gold, sum over batch
            fsum = wp.tile([1, BC], F32, tag="fsum")
            nc.vector.scalar_tensor_tensor(
                out=fsum[:], in0=flog[:], scalar=0.0, in1=slogsum[:], op0=ALU.add, op1=ALU.add,
            )
            nc.vector.scalar_tensor_tensor(
                out=fsum[:], in0=fsum[:], scalar=0.0, in1=ssum[:], op0=ALU.add, op1=ALU.add,
            )
            nc.vector.scalar_tensor_tensor(
                out=fsum[:], in0=fsum[:], scalar=0.0, in1=gold_sb[:], op0=ALU.add, op1=ALU.subtract,
            )
            lp_t = wp.tile([1, 1], F32, tag="lp")
            nc.vector.tensor_reduce(out=lp_t[:], in_=fsum[:], axis=AX.X, op=ALU.add)
            nc.sync.dma_start(out=d_out[:], in_=lp_t[:])
            ps_p5.__exit__(None, None, None)

    nc.finalize()
    return nc


def _prep_inmaps(inputs):
    bf = ml_dtypes.bfloat16
    sent = np.asarray(inputs["sentences"])
    tags = np.asarray(inputs["tags"])
    embed = np.asarray(inputs["embed"], dtype=np.float32)
    trans = np.asarray(inputs["transitions"], dtype=np.float32)
    h0 = np.asarray(inputs["h0"], dtype=np.float32)
    c0 = np.asarray(inputs["c0"], dtype=np.float32)
    W_out = np.asarray(inputs["W_out"], dtype=np.float32)
    b_out = np.asarray(inputs["b_out"], dtype=np.float32)

    rs = np.full((G, 1), 0.5, np.float32)
    rs[2 * H : 3 * H] = 1.0  # g-gate rows unscaled

    embed_bf = np.ascontiguousarray(embed.astype(bf))

    def chunk_weights(W):  # W [G, K_in] -> [128, 2, CH, 128] = [p, k, c, m]
        Kin = W.shape[1]
        Wr = W.reshape(4, 2, 128, Kin // 128, 128)  # [gate, hh, m, k, p]
        return np.ascontiguousarray(Wr.transpose(4, 3, 0, 1, 2).reshape(128, Kin // 128, CH, 128))

    wih = np.zeros((128, 2, 2, CH, 128), np.float32)
    whh = np.zeros((128, 2, 2, CH, 128), np.float32)
    xbias = np.zeros((128, 2, CH), np.float32)
    for d, (Wih, Whh, b) in enumerate(
        [
            (inputs["Wih_f"], inputs["Whh_f"], inputs["b_f"]),
            (inputs["Wih_b"], inputs["Whh_b"], inputs["b_b"]),
        ]
    ):
        Wih = np.asarray(Wih, np.float32) * rs
        Whh = np.asarray(Whh, np.float32) * rs * 0.5
        bt = np.asarray(b, np.float32) * rs[:, 0]
        wih[:, d] = chunk_weights(Wih)
        whh[:, d] = chunk_weights(Whh)
        xbias[:, d] = bt.reshape(4, 2, 128).transpose(2, 0, 1).reshape(128, CH)
    wih = np.ascontiguousarray(wih.astype(bf))
    whh = np.ascontiguousarray(whh.astype(bf))

    # wout^T [p, d, k, m] = 0.5 * W_out[m, d*256 + k*128 + p]
    wout = np.ascontiguousarray(
        (0.5 * W_out).reshape(C, 2, 2, 128).transpose(3, 1, 2, 0).astype(bf)
    )
    bout = np.ascontiguousarray(b_out[:, None])
    transT = np.ascontiguousarray(trans.T)
    tstop = np.ascontiguousarray(trans[STOP, :][:, None])

    in_maps = []
    for q in range(NCORES):
        bs = slice(q * BC, (q + 1) * BC)
        sq = sent[bs]  # [BC, L]
        tq = tags[bs]
        idx_f = np.ascontiguousarray(
            sq.T.reshape(NT, TPT, BC).transpose(1, 2, 0).reshape(128, NT).astype(np.int32)
        )
        sqr = sq[:, ::-1]
        idx_b = np.ascontiguousarray(
            sqr.T.reshape(NT, TPT, BC).transpose(1, 2, 0).reshape(128, NT).astype(np.int32)
        )
        h0q = np.ascontiguousarray(
            (2.0 * h0[:, bs, :]).reshape(2, BC, 2, 128).transpose(3, 0, 2, 1).astype(bf)
        )
        c0q = np.ascontiguousarray(
            (2.0 * c0[:, bs, :]).reshape(2, BC, 2, 128).transpose(3, 0, 2, 1).astype(np.float32)
        )
        te_prev = np.concatenate(
            [np.full((BC, 1), START, tags.dtype), tq[:, :-1]], axis=1
        )  # prev tag at each t
        ar = np.arange(C)
        ohprev = (ar[:, None, None] == te_prev[None, :, :]).astype(np.float32)
        ohcur = (ar[:, None, None] == tq[None, :, :]).astype(np.float32)
        ohlast = (ar[:, None] == tq[None, :, L - 1]).astype(np.float32)
        a0 = (ar[:, None] == START).astype(np.float32) * np.ones((1, BC), np.float32)
        in_maps.append(
            {
                "embed_bf": embed_bf,
                "idx_f": idx_f,
                "idx_b": idx_b,
                "wih": wih,
                "whh": whh,
                "xbias": xbias,
                "h0T": h0q,
                "c0T": c0q,
                "woutT": wout,
                "bout": bout,
                "transT": transT,
                "tstop": tstop,
                "ohprev": np.ascontiguousarray(ohprev),
                "ohcur": np.ascontiguousarray(ohcur),
                "ohlast": np.ascontiguousarray(ohlast),
                "a0": np.ascontiguousarray(a0),
            }
        )
    return in_maps


def get_module():
    if "nc" not in _CACHE:
        _CACHE["nc"] = _build_module()
    return _CACHE["nc"]


def kernel(**inputs):
    nc = get_module()
    in_maps = _prep_inmaps(inputs)
    res = run_bass_kernel_spmd(nc, in_maps, core_ids=list(range(NCORES)))
    total = sum(float(r["out"][0, 0]) for r in res.results)
    return np.float32(total / B)



# revision 5
# speedup vs baseline: 1.2763x; 1.2763x over previous
"""BiLSTM-CRF loss kernel for 8 Trainium2 NeuronCores (v2).

Sharding: data-parallel over batch (64 -> 8 cores x 8 rows). Each core runs
both LSTM directions for its batch shard, computes CRF emissions, runs the
CRF forward pass in the exp domain, the gold-path score, and writes the
partial sum of (forward - gold) over its 8 rows. Host sums partials / 64.

v2 changes vs v1:
  - Single gather/transpose set: backward direction consumes forward-time
    xpT at reversed indices and writes hsT at reversed slots, so hs_b ends
    up in forward-time order for the emission matmuls.
  - Input projection xp is injected into the LSTM gate PSUM via an
    identity-stationary matmul, eliminating the per-step DVE add; tanh
    reads PSUM directly on the Activation engine.
  - The v gate product runs on GpSimd, balancing DVE.
  - P1 bias-adds and P3's exp(feats) run on the Activation engine with
    per-partition bias (bout folded into the exp).
  - No max-shift in the CRF: E = exp(W h + b) directly; periodic renorm
    keeps the exp-domain scan in f32 range.
  - CRF scan in bf16 (single HW matmul per step instead of an fp32 pair),
    split into two staggered 4-row chains with renorm every 16 steps.
"""

import sys

sys.path.insert(0, "/opt/trn_rl_repo")

import numpy as np
import ml_dtypes

import concourse.bass as bass
from concourse import bacc
import concourse.tile as tile
from concourse import mybir
from concourse import bass_isa
from concourse.bass import IndirectOffsetOnAxis
from concourse.bass_utils import run_bass_kernel_spmd
from concourse.masks import make_identity

F32 = mybir.dt.float32
BF16 = mybir.dt.bfloat16
I32 = mybir.dt.int32
ALU = mybir.AluOpType
AF = mybir.ActivationFunctionType
AX = mybir.AxisListType

B, L, E, H, C = 64, 256, 256, 256, 20
G = 4 * H
NCORES = 8
BC = B // NCORES            # batch rows per core
CH = 8                      # gate-hidden chunks of 128 (c = gate*2 + half)
NT = (L * BC) // 128        # token tiles = 16
TPT = 128 // BC             # timesteps per token tile = 16
REN = 16                    # CRF renorm interval (per chain)
NREN = L // REN             # 16
HBC = BC // 2               # rows per CRF chain = 4
START, STOP = 18, 19

_CACHE = {}
DEBUG = False


def _build_module():
    nc = bacc.Bacc(None, target_bir_lowering=False, debug=False)

    # ---- DRAM I/O ----
    d_embed = nc.dram_tensor("embed_bf", [50000, E], BF16, kind="ExternalInput")
    d_idxf = nc.dram_tensor("idx_f", [128, NT], I32, kind="ExternalInput")
    d_wih = nc.dram_tensor("wih", [128, 2, 2, CH, 128], BF16, kind="ExternalInput")
    d_whh = nc.dram_tensor("whh", [128, 2, 2, CH, 128], BF16, kind="ExternalInput")
    d_xbias = nc.dram_tensor("xbias", [128, 2, CH], F32, kind="ExternalInput")
    d_h0 = nc.dram_tensor("h0T", [128, 2, 2, BC], BF16, kind="ExternalInput")
    d_c0 = nc.dram_tensor("c0T", [128, 2, 2, BC], F32, kind="ExternalInput")
    d_wout = nc.dram_tensor("woutT", [128, 2, 2, C], BF16, kind="ExternalInput")
    d_bout = nc.dram_tensor("bout", [C, 1], F32, kind="ExternalInput")
    d_transT = nc.dram_tensor("transT", [C, C], F32, kind="ExternalInput")
    d_transTb = nc.dram_tensor("transTb", [C, C], BF16, kind="ExternalInput")
    d_tstop = nc.dram_tensor("tstop", [C, 1], F32, kind="ExternalInput")
    d_ohprev = nc.dram_tensor("ohprev", [C, BC, L], BF16, kind="ExternalInput")
    d_ohcur = nc.dram_tensor("ohcur", [C, BC, L], F32, kind="ExternalInput")
    d_ohcur_tb = nc.dram_tensor("ohcur_tb", [C, L, BC], F32, kind="ExternalInput")
    d_ohlast = nc.dram_tensor("ohlast", [C, BC], F32, kind="ExternalInput")
    d_a0 = nc.dram_tensor("a0", [C, BC], BF16, kind="ExternalInput")
    d_gold = nc.dram_tensor("gold_out", [1, BC], F32, kind="ExternalOutput")
    d_paf = nc.dram_tensor("paf_out", [1, BC], F32, kind="ExternalOutput")
    d_sall = nc.dram_tensor("sall_out", [1, BC, NREN], F32, kind="ExternalOutput")
    if DEBUG:
        d_dbg_eT = nc.dram_tensor("dbg_eT", [C, L, BC], F32, kind="ExternalOutput")
        d_dbg_gsum = nc.dram_tensor("dbg_gsum", [C, BC], F32, kind="ExternalOutput")
        d_dbg_gold = nc.dram_tensor("dbg_gold", [1, BC], F32, kind="ExternalOutput")
        d_dbg_hf = nc.dram_tensor("dbg_hf", [128, 2, BC], BF16, kind="ExternalOutput")
        d_dbg_hb = nc.dram_tensor("dbg_hb", [128, 2, BC], BF16, kind="ExternalOutput")
        d_dbg_sall = nc.dram_tensor("dbg_sall", [1, BC, NREN], F32, kind="ExternalOutput")
        d_dbg_xp = nc.dram_tensor("dbg_xp", [128, 8, CH, BC], BF16, kind="ExternalOutput")
        d_dbg_cst = nc.dram_tensor("dbg_cst", [128, 2, 2, BC], F32, kind="ExternalOutput")

    with tile.TileContext(nc) as tc:
        with (
            tc.tile_pool(name="persist", bufs=1) as pp,
            tc.tile_pool(name="work", bufs=3) as wp,
            tc.tile_pool(name="lstm", bufs=3) as lp,
        ):
            # ---- persistent SBUF ----
            wih_sb = pp.tile([128, 2, 2, CH, 128], BF16, tag="wih")
            whh_sb = pp.tile([128, 2, 2, CH, 128], BF16, tag="whh")
            xbias_sb = pp.tile([128, 2, CH], F32, tag="xbias")
            wout_sb = pp.tile([128, 2, 2, C], BF16, tag="wout")
            bout_sb = pp.tile([C, 1], F32, tag="bout")
            transT_sb = pp.tile([C, C], F32, tag="transT")
            transTb_sb = pp.tile([C, C], BF16, tag="transTb")
            tstop_sb = pp.tile([C, 1], F32, tag="tstop")
            ohprev_sb = pp.tile([C, BC, L], BF16, tag="ohprev")
            ohcur_sb = pp.tile([C, BC, L], F32, tag="ohcur")
            ohcur_tb_sb = pp.tile([C, L, BC], F32, tag="ohcur_tb")
            ohlast_sb = pp.tile([C, BC], F32, tag="ohlast")
            idxf_sb = pp.tile([128, NT], I32, tag="idxf")
            ident128 = pp.tile([128, 128], BF16, tag="id128")
            xTf = pp.tile([128, 2, NT, 128], BF16, tag="xTf")
            # xp^T: [ghid-part, t, chunk, b]  (both dirs in forward time order)
            xpT = [pp.tile([128, L, CH, BC], BF16, name=f"xpT{d}", tag=f"xpT{d}") for d in (0, 1)]
            # H history: [hid-part, k-half, slot(0..L), b]
            # dir0: init slot 0, step s reads s, writes s+1 (h_f[t] at slot t+1)
            # dir1: init slot L, step s reads L-s, writes L-1-s (h_b[t] at slot t)
            hsT = [pp.tile([128, 2, L + 1, BC], BF16, name=f"hsT{d}", tag=f"hsT{d}") for d in (0, 1)]
            cst = [pp.tile([128, 2, BC], F32, name=f"cst{d}", tag=f"cst{d}") for d in (0, 1)]
            eT = pp.tile([C, L, BC], F32, tag="eT")
            pplus = pp.tile([C, C], BF16, tag="pplus")
            wstop = pp.tile([C, 1], BF16, tag="wstop")
            # applied renorm scales (exact f32 bookkeeping): [1, chain*HBC, NREN]
            sall = pp.tile([1, BC, NREN], F32, tag="sall")
            ones1 = pp.tile([1, C], F32, tag="ones1")
            ones20c = pp.tile([C, 1], BF16, tag="ones20c")
            ones20f = pp.tile([C, 1], F32, tag="ones20f")
            avec = [pp.tile([C, 2, HBC], BF16, name=f"avec{x}", tag=f"avec{x}") for x in (0, 1)]
            gsum = pp.tile([C, BC], F32, tag="gsum")
            gold_sb = pp.tile([1, BC], F32, tag="gold")
            cnt_sb = pp.tile([C, BC], F32, tag="cnt")

            # ---- load constants ----
            nc.sync.dma_start(out=wih_sb[:], in_=d_wih[:])
            nc.sync.dma_start(out=whh_sb[:], in_=d_whh[:])
            nc.sync.dma_start(out=xbias_sb[:], in_=d_xbias[:])
            nc.sync.dma_start(out=wout_sb[:], in_=d_wout[:])
            nc.sync.dma_start(out=bout_sb[:], in_=d_bout[:])
            nc.sync.dma_start(out=transT_sb[:], in_=d_transT[:])
            nc.sync.dma_start(out=transTb_sb[:], in_=d_transTb[:])
            nc.sync.dma_start(out=tstop_sb[:], in_=d_tstop[:])
            nc.sync.dma_start(out=ohprev_sb[:], in_=d_ohprev[:])
            nc.sync.dma_start(out=ohcur_sb[:], in_=d_ohcur[:])
            nc.sync.dma_start(out=ohcur_tb_sb[:], in_=d_ohcur_tb[:])
            nc.sync.dma_start(out=ohlast_sb[:], in_=d_ohlast[:])
            nc.sync.dma_start(out=idxf_sb[:], in_=d_idxf[:])
            nc.sync.dma_start(out=hsT[0][:, :, 0, :], in_=d_h0[:, 0, :, :])
            nc.sync.dma_start(out=hsT[1][:, :, L, :], in_=d_h0[:, 1, :, :])
            for d in (0, 1):
                nc.sync.dma_start(out=cst[d][:], in_=d_c0[:, d, :, :])
            make_identity(nc, ident128[:])
            nc.vector.memset(ones1[:], 1.0)
            nc.vector.memset(ones20c[:], 1.0)
            nc.vector.memset(ones20f[:], 1.0)

            # P+ = exp(transT) in bf16;  wstop = exp(T[STOP,:]) in bf16
            nc.scalar.activation(pplus[:], transT_sb[:], AF.Exp)
            nc.scalar.activation(wstop[:], tstop_sb[:], AF.Exp)

            # A0 = onehot(START): chain 0 rows 0..3, chain 1 rows 4..7
            for x in (0, 1):
                nc.vector.memset(avec[x][:], 0.0)
                nc.sync.dma_start(
                    out=avec[x][:, 0, :], in_=d_a0[:, x * HBC : (x + 1) * HBC]
                )

            # ---- gold transition score (inputs only; runs during P1) ----
            # pu = trans @ ohprev ; gsum = sum_t (pu * ohcur)
            ps_gold = tc.tile_pool(name="ps_gold", bufs=1, space="PSUM")
            psG = ps_gold.__enter__()
            pu = psG.tile([C, BC * L], F32, tag="pu")
            for n in range(4):
                nc.tensor.matmul(
                    pu[:, n * 512 : (n + 1) * 512],
                    transTb_sb[:],
                    ohprev_sb[:].rearrange("p b t -> p (b t)")[
                        :, n * 512 : (n + 1) * 512
                    ],
                    start=True,
                    stop=True,
                )
            prod = pp.tile([C, BC, L], F32, tag="prod")
            nc.vector.scalar_tensor_tensor(
                out=prod[:].rearrange("p b t -> p (b t)"), in0=pu[:], scalar=0.0,
                in1=ohcur_sb[:].rearrange("p b t -> p (b t)"), op0=ALU.add, op1=ALU.mult,
            )
            nc.vector.tensor_reduce(out=gsum[:], in_=prod[:], axis=AX.X, op=ALU.add)
            # cnt[c,b] = #timesteps with tag c (for the bout term of emissions)
            nc.vector.tensor_reduce(out=cnt_sb[:], in_=ohcur_sb[:], axis=AX.X, op=ALU.add)
            # cnt *= bout (per-tag emission bias counts); gsum += cnt
            nc.gpsimd.tensor_tensor(
                out=cnt_sb[:], in0=cnt_sb[:],
                in1=bout_sb[:].to_broadcast([C, BC]), op=ALU.mult,
            )
            nc.gpsimd.tensor_add(gsum[:], gsum[:], cnt_sb[:])
            ps_gold.__exit__(None, None, None)

            # ---- P1: gather + transpose + input projection ----
            ps_p1 = tc.tile_pool(name="ps_p1", bufs=2, space="PSUM")
            psA = ps_p1.__enter__()
            for ti in range(NT):
                gx = wp.tile([128, E], BF16, tag="gx")
                nc.gpsimd.indirect_dma_start(
                    out=gx[:],
                    out_offset=None,
                    in_=d_embed[:],
                    in_offset=IndirectOffsetOnAxis(ap=idxf_sb[:, ti : ti + 1], axis=0),
                )
                for k in (0, 1):
                    pt = psA.tile([128, 128], BF16, tag="ptr")
                    nc.tensor.transpose(
                        pt[:], gx[:, k * 128 : (k + 1) * 128], ident128[:]
                    )
                    nc.vector.tensor_copy(xTf[:, k, ti, :], pt[:])
            for d in (0, 1):
                for c in range(CH):
                    for h in (0, 1):
                        pj = psA.tile([128, 1024], F32, tag="pj", bufs=3)
                        for k in (0, 1):
                            for nb in (0, 1):
                                nc.tensor.matmul(
                                    pj[:, nb * 512 : (nb + 1) * 512],
                                    wih_sb[:, d, k, c, :],
                                    xTf[:, k, h * 8 + nb * 4 : h * 8 + (nb + 1) * 4, :],
                                    start=(k == 0),
                                    stop=(k == 1),
                                )
                        # xp^T[t, c, b] = pj + bias (split across ACT and DVE)
                        if (c + h) % 2 == 0:
                            nc.scalar.activation(
                                out=xpT[d][:, h * 128 : (h + 1) * 128, c, :],
                                in_=pj[:].rearrange("p (tt b) -> p tt b", b=BC),
                                func=AF.Identity,
                                bias=xbias_sb[:, d, c : c + 1],
                            )
                        else:
                            nc.vector.tensor_scalar(
                                out=xpT[d][:, h * 128 : (h + 1) * 128, c, :],
                                in0=pj[:].rearrange("p (tt b) -> p tt b", b=BC),
                                scalar1=xbias_sb[:, d, c : c + 1],
                                scalar2=None,
                                op0=ALU.add,
                            )
            ps_p1.__exit__(None, None, None)

            # ---- P2: LSTM recurrence (both directions interleaved) ----
            ps_p2 = tc.tile_pool(name="ps_p2", bufs=4, space="PSUM")
            psB = ps_p2.__enter__()
            for s in range(L):
                for d in (0, 1):
                    rs = s if d == 0 else L - s          # read slot
                    ws = s + 1 if d == 0 else L - 1 - s  # write slot
                    xs = s if d == 0 else L - 1 - s      # xpT time index
                    pg = psB.tile([128, CH, BC], F32, tag="pg")
                    # inject xp first (moving=xpT slice, stationary=identity)
                    nc.tensor.matmul(
                        pg[:].rearrange("p c b -> p (c b)"),
                        ident128[:],
                        xpT[d][:, xs, :, :].rearrange("p c b -> p (c b)"),
                        start=True,
                        stop=False,
                        skip_group_check=True,
                    )
                    for c in range(CH):
                        for k in (0, 1):
                            nc.tensor.matmul(
                                pg[:, c, :],
                                whh_sb[:, d, k, c, :],
                                hsT[d][:, k, rs, :],
                                start=False,
                                stop=(c == CH - 1 and k == 1),
                                skip_group_check=True,
                            )
                    th = lp.tile([128, CH, BC], F32, tag="th")
                    nc.scalar.activation(th[:], pg[:], AF.Tanh)
                    u = lp.tile([128, 2, BC], F32, tag="u")
                    nc.vector.scalar_tensor_tensor(
                        out=u[:], in0=th[:, 0:2, :], scalar=1.0, in1=th[:, 4:6, :],
                        op0=ALU.add, op1=ALU.mult,
                    )
                    v = lp.tile([128, 2, BC], F32, tag="v")
                    nc.vector.scalar_tensor_tensor(
                        out=v[:], in0=th[:, 2:4, :], scalar=1.0, in1=cst[d][:],
                        op0=ALU.add, op1=ALU.mult,
                    )
                    nc.vector.scalar_tensor_tensor(
                        out=cst[d][:], in0=v[:], scalar=0.5, in1=u[:],
                        op0=ALU.mult, op1=ALU.add,
                    )
                    tcc = lp.tile([128, 2, BC], F32, tag="tcc")
                    nc.scalar.activation(tcc[:], cst[d][:], AF.Tanh, scale=0.5)
                    nc.vector.scalar_tensor_tensor(
                        out=hsT[d][:, :, ws, :], in0=th[:, 6:8, :], scalar=1.0,
                        in1=tcc[:], op0=ALU.add, op1=ALU.mult,
                    )
            ps_p2.__exit__(None, None, None)

            # ---- P3: emissions E = exp(sum_d Wout_d @ H_d + bout) ----
            ps_p3 = tc.tile_pool(name="ps_p3", bufs=1, space="PSUM")
            psC = ps_p3.__enter__()
            pf = psC.tile([C, L * BC], F32, tag="pf")
            for d in (0, 1):
                for k in (0, 1):
                    for n in range(4):
                        base = 1 + n * 64 if d == 0 else n * 64
                        nc.tensor.matmul(
                            pf[:, n * 512 : (n + 1) * 512],
                            wout_sb[:, d, k, :],
                            hsT[d][:, k, base : base + 64, :],
                            start=(d == 0 and k == 0),
                            stop=(d == 1 and k == 1),
                        )
            nc.scalar.activation(
                out=eT[:].rearrange("p t b -> p (t b)"),
                in_=pf[:],
                func=AF.Exp,
                bias=bout_sb[:, 0:1],
            )
            # gold emissions: sum_t pf[gold tag] (bout term already via cnt)
            prod2 = pp.tile([C, L, BC], F32, tag="prod2")
            nc.vector.scalar_tensor_tensor(
                out=prod2[:].rearrange("p t b -> p (t b)"), in0=pf[:], scalar=0.0,
                in1=ohcur_tb_sb[:].rearrange("p t b -> p (t b)"), op0=ALU.add, op1=ALU.mult,
            )
            gsum2 = pp.tile([C, BC], F32, tag="gsum2")
            nc.vector.tensor_reduce(
                out=gsum2[:],
                in_=prod2[:].rearrange("p t b -> p b t"),
                axis=AX.X, op=ALU.add,
            )
            nc.gpsimd.tensor_add(gsum[:], gsum[:], gsum2[:])
            # + T[STOP, tag_last]
            stopterm = pp.tile([C, BC], F32, tag="stopterm")
            nc.gpsimd.tensor_tensor(
                out=stopterm[:], in0=ohlast_sb[:],
                in1=tstop_sb[:].to_broadcast([C, BC]), op=ALU.mult,
            )
            nc.gpsimd.tensor_add(gsum[:], gsum[:], stopterm[:])
            ps_p3.__exit__(None, None, None)

            # ---- P5: CRF forward scan, two staggered 4-row chains ----
            ps_p5 = tc.tile_pool(name="ps_p5", bufs=2, space="PSUM")
            psD = ps_p5.__enter__()
            # gold reduce over tags (PE while idle-ish): gold = ones20c^T @ gsum
            pgold = psD.tile([1, BC], F32, tag="pgold", bufs=1)
            nc.tensor.matmul(pgold[:], ones20f[:], gsum[:], start=True, stop=True)
            nc.vector.tensor_copy(gold_sb[:], pgold[:])

            for t in range(L):
                for x in (0, 1):
                    bs = slice(x * HBC, (x + 1) * HBC)
                    pa = psD.tile([C, HBC], F32, tag=f"pa{x}")
                    nc.tensor.matmul(
                        pa[:], pplus[:], avec[x][:, t % 2, :], start=True, stop=True
                    )
                    nc.vector.scalar_tensor_tensor(
                        out=avec[x][:, (t + 1) % 2, :], in0=pa[:], scalar=0.0,
                        in1=eT[:, t, bs], op0=ALU.add, op1=ALU.mult,
                    )
                    # staggered renorm: chain 0 at t%16==15, chain 1 at t%16==7
                    if t % REN == (REN - 1 if x == 0 else REN // 2 - 1):
                        rn = t // REN
                        cur = (t + 1) % 2
                        ssum = wp.tile([C, HBC], F32, tag=f"ssum{x}")
                        nc.gpsimd.partition_all_reduce(
                            ssum[:], avec[x][:, cur, :], channels=C,
                            reduce_op=bass_isa.ReduceOp.add,
                        )
                        srec = wp.tile([C, HBC], F32, tag=f"srec{x}")
                        nc.vector.reciprocal(srec[:], ssum[:])
                        # record the applied scale (exact bookkeeping)
                        nc.vector.tensor_copy(sall[0:1, bs, rn], srec[0:1, :])
                        nc.vector.scalar_tensor_tensor(
                            out=avec[x][:, cur, :],
                            in0=avec[x][:, cur, :], scalar=0.0,
                            in1=srec[:], op0=ALU.add, op1=ALU.mult,
                        )

            # ---- P6: ship gold/paf/sall to host (lns done in float64 there;
            # the ACT Ln table is invalid for the ~1e-24 srec magnitudes) ----
            paf = psD.tile([1, BC], F32, tag="paf", bufs=1)
            for x in (0, 1):
                nc.tensor.matmul(
                    paf[:, x * HBC : (x + 1) * HBC], wstop[:],
                    avec[x][:, L % 2, :], start=True, stop=True,
                )
            paf_sb = wp.tile([1, BC], F32, tag="paf_sb")
            nc.vector.tensor_copy(paf_sb[:], paf[:])
            nc.sync.dma_start(out=d_gold[:], in_=gold_sb[:])
            nc.sync.dma_start(out=d_paf[:], in_=paf_sb[:])
            nc.sync.dma_start(out=d_sall[:], in_=sall[:])
            if DEBUG:
                nc.sync.dma_start(out=d_dbg_eT[:], in_=eT[:])
                nc.sync.dma_start(out=d_dbg_gsum[:], in_=gsum[:])
                nc.sync.dma_start(out=d_dbg_gold[:], in_=gold_sb[:])
                nc.sync.dma_start(out=d_dbg_hf[:], in_=hsT[0][:, :, L, :])
                nc.sync.dma_start(out=d_dbg_hb[:], in_=hsT[1][:, :, 0, :])
                nc.sync.dma_start(out=d_dbg_sall[:], in_=sall[:])
                nc.sync.dma_start(out=d_dbg_xp[:], in_=xpT[0][:, 0:8, :, :])
                for d in (0, 1):
                    nc.sync.dma_start(out=d_dbg_cst[:, d, :, :], in_=cst[d][:])
            ps_p5.__exit__(None, None, None)

    nc.finalize()
    return nc


def _prep_inmaps(inputs):
    bf = ml_dtypes.bfloat16
    sent = np.asarray(inputs["sentences"])
    tags = np.asarray(inputs["tags"])
    embed = np.asarray(inputs["embed"], dtype=np.float32)
    trans = np.asarray(inputs["transitions"], dtype=np.float32)
    h0 = np.asarray(inputs["h0"], dtype=np.float32)
    c0 = np.asarray(inputs["c0"], dtype=np.float32)
    W_out = np.asarray(inputs["W_out"], dtype=np.float32)
    b_out = np.asarray(inputs["b_out"], dtype=np.float32)

    rs = np.full((G, 1), 0.5, np.float32)
    rs[2 * H : 3 * H] = 1.0  # g-gate rows unscaled

    embed_bf = np.ascontiguousarray(embed.astype(bf))

    def chunk_weights(W):  # W [G, K_in] -> [128, 2, CH, 128] = [p, k, c, m]
        Kin = W.shape[1]
        Wr = W.reshape(4, 2, 128, Kin // 128, 128)  # [gate, hh, m, k, p]
        return np.ascontiguousarray(Wr.transpose(4, 3, 0, 1, 2).reshape(128, Kin // 128, CH, 128))

    wih = np.zeros((128, 2, 2, CH, 128), np.float32)
    whh = np.zeros((128, 2, 2, CH, 128), np.float32)
    xbias = np.zeros((128, 2, CH), np.float32)
    for d, (Wih, Whh, b) in enumerate(
        [
            (inputs["Wih_f"], inputs["Whh_f"], inputs["b_f"]),
            (inputs["Wih_b"], inputs["Whh_b"], inputs["b_b"]),
        ]
    ):
        Wih = np.asarray(Wih, np.float32) * rs
        Whh = np.asarray(Whh, np.float32) * rs * 0.5
        bt = np.asarray(b, np.float32) * rs[:, 0]
        wih[:, d] = chunk_weights(Wih)
        whh[:, d] = chunk_weights(Whh)
        xbias[:, d] = bt.reshape(4, 2, 128).transpose(2, 0, 1).reshape(128, CH)
    wih = np.ascontiguousarray(wih.astype(bf))
    whh = np.ascontiguousarray(whh.astype(bf))

    # wout^T [p, d, k, m] = 0.5 * W_out[m, d*256 + k*128 + p]
    wout = np.ascontiguousarray(
        (0.5 * W_out).reshape(C, 2, 2, 128).transpose(3, 1, 2, 0).astype(bf)
    )
    bout = np.ascontiguousarray(b_out[:, None])
    transT = np.ascontiguousarray(trans.T)
    transTb = np.ascontiguousarray(trans.T.astype(bf))
    tstop = np.ascontiguousarray(trans[STOP, :][:, None])

    in_maps = []
    for q in range(NCORES):
        bs = slice(q * BC, (q + 1) * BC)
        sq = sent[bs]  # [BC, L]
        tq = tags[bs]
        idx_f = np.ascontiguousarray(
            sq.T.reshape(NT, TPT, BC).transpose(1, 2, 0).reshape(128, NT).astype(np.int32)
        )
        h0q = np.ascontiguousarray(
            (2.0 * h0[:, bs, :]).reshape(2, BC, 2, 128).transpose(3, 0, 2, 1).astype(bf)
        )
        c0q = np.ascontiguousarray(
            (2.0 * c0[:, bs, :]).reshape(2, BC, 2, 128).transpose(3, 0, 2, 1).astype(np.float32)
        )
        te_prev = np.concatenate(
            [np.full((BC, 1), START, tags.dtype), tq[:, :-1]], axis=1
        )  # prev tag at each t
        ar = np.arange(C)
        ohprev = (ar[:, None, None] == te_prev[None, :, :]).astype(np.float32)
        ohcur = (ar[:, None, None] == tq[None, :, :]).astype(np.float32)
        ohcur_tb = np.ascontiguousarray(ohcur.transpose(0, 2, 1))
        ohlast = (ar[:, None] == tq[None, :, L - 1]).astype(np.float32)
        a0 = ((ar[:, None] == START) * np.ones((1, BC))).astype(bf)
        in_maps.append(
            {
                "embed_bf": embed_bf,
                "idx_f": idx_f,
                "wih": wih,
                "whh": whh,
                "xbias": xbias,
                "h0T": h0q,
                "c0T": c0q,
                "woutT": wout,
                "bout": bout,
                "transT": transT,
                "transTb": transTb,
                "tstop": tstop,
                "ohprev": np.ascontiguousarray(ohprev.astype(bf)),
                "ohcur": np.ascontiguousarray(ohcur),
                "ohcur_tb": ohcur_tb,
                "ohlast": np.ascontiguousarray(ohlast),
                "a0": np.ascontiguousarray(a0),
            }
        )
    return in_maps


def get_module():
    if "nc" not in _CACHE:
        _CACHE["nc"] = _build_module()
    return _CACHE["nc"]


def _finalize(outs):
    """Host-side: partial = sum_b [ln(paf_b) - sum_r ln(srec_br) - gold_b]."""
    paf = np.asarray(outs["paf_out"], np.float64)[0]
    sall = np.asarray(outs["sall_out"], np.float64)[0]
    gold = np.asarray(outs["gold_out"], np.float64)[0]
    F = np.log(paf) - np.log(sall).sum(axis=1)
    return float((F - gold).sum())


def kernel(**inputs):
    nc = get_module()
    in_maps = _prep_inmaps(inputs)
    res = run_bass_kernel_spmd(nc, in_maps, core_ids=list(range(NCORES)))
    total = sum(_finalize(r) for r in res.results)
    return np.float32(total / B)


# revision 6
# speedup vs baseline: 2.1180x; 1.6595x over previous
"""BiLSTM-CRF loss kernel for 8 Trainium2 NeuronCores (v2).

Sharding: data-parallel over batch (64 -> 8 cores x 8 rows). Each core runs
both LSTM directions for its batch shard, computes CRF emissions, runs the
CRF forward pass in the exp domain, the gold-path score, and writes the
partial sum of (forward - gold) over its 8 rows. Host sums partials / 64.

v2 changes vs v1:
  - Single gather/transpose set: backward direction consumes forward-time
    xpT at reversed indices and writes hsT at reversed slots, so hs_b ends
    up in forward-time order for the emission matmuls.
  - Input projection xp is injected into the LSTM gate PSUM via an
    identity-stationary matmul, eliminating the per-step DVE add; tanh
    reads PSUM directly on the Activation engine.
  - The v gate product runs on GpSimd, balancing DVE.
  - P1 bias-adds and P3's exp(feats) run on the Activation engine with
    per-partition bias (bout folded into the exp).
  - No max-shift in the CRF: E = exp(W h + b) directly; periodic renorm
    keeps the exp-domain scan in f32 range.
  - CRF scan in bf16 (single HW matmul per step instead of an fp32 pair),
    split into two staggered 4-row chains with renorm every 16 steps.
"""

import sys

sys.path.insert(0, "/opt/trn_rl_repo")

import numpy as np
import ml_dtypes

import concourse.bass as bass
from concourse import bacc
import concourse.tile as tile
from concourse import mybir
from concourse import bass_isa
from concourse.bass import IndirectOffsetOnAxis
from concourse.bass_utils import run_bass_kernel_spmd
from concourse.masks import make_identity

F32 = mybir.dt.float32
BF16 = mybir.dt.bfloat16
I32 = mybir.dt.int32
ALU = mybir.AluOpType
AF = mybir.ActivationFunctionType
AX = mybir.AxisListType

B, L, E, H, C = 64, 256, 256, 256, 20
G = 4 * H
NCORES = 8
BC = B // NCORES            # batch rows per core
CH = 8                      # gate-hidden chunks of 128 (c = gate*2 + half)
NT = (L * BC) // 128        # token tiles = 16
TPT = 128 // BC             # timesteps per token tile = 16
REN = 16                    # CRF renorm interval (per chain)
NREN = L // REN             # 16
HBC = BC // 2               # rows per CRF chain = 4
START, STOP = 18, 19

_CACHE = {}
DEBUG = False
SPEC_W = 12    # LSTM speculative warmup steps
CSPEC_W = 12   # CRF speculative warmup steps
NH = L // 2     # 128


def _build_module():
    nc = bacc.Bacc(None, target_bir_lowering=False, debug=False)

    # ---- DRAM I/O ----
    d_embed = nc.dram_tensor("embed_bf", [50000, E], BF16, kind="ExternalInput")
    d_idxf = nc.dram_tensor("idx_f", [128, NT], I32, kind="ExternalInput")
    d_wih = nc.dram_tensor("wih", [128, 2, 2, CH, 128], BF16, kind="ExternalInput")
    d_whh = nc.dram_tensor("whh", [128, 2, 2, CH, 128], BF16, kind="ExternalInput")
    d_xbias = nc.dram_tensor("xbias", [128, 2, CH], F32, kind="ExternalInput")
    d_h0 = nc.dram_tensor("h0T", [128, 2, 2, BC], BF16, kind="ExternalInput")
    d_c0 = nc.dram_tensor("c0T", [128, 2, 2, BC], F32, kind="ExternalInput")
    d_wout = nc.dram_tensor("woutT", [128, 2, 2, C], BF16, kind="ExternalInput")
    d_bout = nc.dram_tensor("bout", [C, 1], F32, kind="ExternalInput")
    d_transT = nc.dram_tensor("transT", [C, C], F32, kind="ExternalInput")
    d_transTb = nc.dram_tensor("transTb", [C, C], BF16, kind="ExternalInput")
    d_tstop = nc.dram_tensor("tstop", [C, 1], F32, kind="ExternalInput")
    d_ohprev = nc.dram_tensor("ohprev", [C, BC, L], BF16, kind="ExternalInput")
    d_ohcur = nc.dram_tensor("ohcur", [C, BC, L], F32, kind="ExternalInput")
    d_ohcur_tb = nc.dram_tensor("ohcur_tb", [C, L, BC], F32, kind="ExternalInput")
    d_ohlast = nc.dram_tensor("ohlast", [C, BC], F32, kind="ExternalInput")
    d_a0 = nc.dram_tensor("a0", [C, BC], BF16, kind="ExternalInput")
    d_gold = nc.dram_tensor("gold_out", [1, BC], F32, kind="ExternalOutput")
    d_paf = nc.dram_tensor("paf_out", [1, BC], F32, kind="ExternalOutput")
    d_sall = nc.dram_tensor("sall_out", [1, BC, NREN], F32, kind="ExternalOutput")
    if DEBUG:
        d_dbg_eT = nc.dram_tensor("dbg_eT", [C, L, BC], F32, kind="ExternalOutput")
        d_dbg_gsum = nc.dram_tensor("dbg_gsum", [C, BC], F32, kind="ExternalOutput")
        d_dbg_gold = nc.dram_tensor("dbg_gold", [1, BC], F32, kind="ExternalOutput")
        d_dbg_hf = nc.dram_tensor("dbg_hf", [128, 2, BC], BF16, kind="ExternalOutput")
        d_dbg_hb = nc.dram_tensor("dbg_hb", [128, 2, BC], BF16, kind="ExternalOutput")
        d_dbg_sall = nc.dram_tensor("dbg_sall", [1, BC, NREN], F32, kind="ExternalOutput")
        d_dbg_xp = nc.dram_tensor("dbg_xp", [128, 8, CH, BC], BF16, kind="ExternalOutput")
        d_dbg_cst = nc.dram_tensor("dbg_cst", [128, 2, 2, BC], F32, kind="ExternalOutput")

    with tile.TileContext(nc) as tc:
        with (
            tc.tile_pool(name="persist", bufs=1) as pp,
            tc.tile_pool(name="work", bufs=3) as wp,
            tc.tile_pool(name="lstm", bufs=3) as lp,
        ):
            # ---- persistent SBUF ----
            wih_sb = pp.tile([128, 2, 2, CH, 128], BF16, tag="wih")
            whh_sb = pp.tile([128, 2, 2, CH, 128], BF16, tag="whh")
            xbias_sb = pp.tile([128, 2, CH], F32, tag="xbias")
            wout_sb = pp.tile([128, 2, 2, C], BF16, tag="wout")
            bout_sb = pp.tile([C, 1], F32, tag="bout")
            transT_sb = pp.tile([C, C], F32, tag="transT")
            transTb_sb = pp.tile([C, C], BF16, tag="transTb")
            tstop_sb = pp.tile([C, 1], F32, tag="tstop")
            ohprev_sb = pp.tile([C, BC, L], BF16, tag="ohprev")
            ohcur_sb = pp.tile([C, BC, L], F32, tag="ohcur")
            ohcur_tb_sb = pp.tile([C, L, BC], F32, tag="ohcur_tb")
            ohlast_sb = pp.tile([C, BC], F32, tag="ohlast")
            idxf_sb = pp.tile([128, NT], I32, tag="idxf")
            ident128 = pp.tile([128, 128], BF16, tag="id128")
            xTf = pp.tile([128, 2, NT, 128], BF16, tag="xTf")
            # xp^T: [ghid-part, t, chunk, b]  (both dirs in forward time order)
            xpT = [pp.tile([128, L, CH, BC], BF16, name=f"xpT{d}", tag=f"xpT{d}") for d in (0, 1)]
            # H history: [hid-part, k-half, slot(0..L), b]
            # dir0: init slot 0, step s reads s, writes s+1 (h_f[t] at slot t+1)
            # dir1: init slot L, step s reads L-s, writes L-1-s (h_b[t] at slot t)
            hsT = [pp.tile([128, 2, L + 1, BC], BF16, name=f"hsT{d}", tag=f"hsT{d}") for d in (0, 1)]
            cst = [[pp.tile([128, 2, BC], F32, name=f"cst{d}{hf}", tag=f"cst{d}{hf}")
                    for hf in (0, 1)] for d in (0, 1)]
            hwu = [pp.tile([128, 2, 2, BC], BF16, name=f"hwu{d}", tag=f"hwu{d}") for d in (0, 1)]
            zro = pp.tile([128, 2, BC], BF16, tag="zro")
            eT = pp.tile([C, L, BC], F32, tag="eT")
            pplus = pp.tile([C, C], BF16, tag="pplus")
            wstop = pp.tile([C, 1], BF16, tag="wstop")
            # applied renorm scales (exact f32 bookkeeping): [1, chain*HBC, NREN]
            sall = pp.tile([1, BC, NREN], F32, tag="sall")
            ones1 = pp.tile([1, C], F32, tag="ones1")
            ones20c = pp.tile([C, 1], BF16, tag="ones20c")
            ones20f = pp.tile([C, 1], F32, tag="ones20f")
            avec = [pp.tile([C, 2, BC], BF16, name=f"avec{x}", tag=f"avec{x}") for x in (0, 1)]
            gsum = pp.tile([C, BC], F32, tag="gsum")
            gold_sb = pp.tile([1, BC], F32, tag="gold")
            cnt_sb = pp.tile([C, BC], F32, tag="cnt")

            # ---- load constants ----
            nc.sync.dma_start(out=wih_sb[:], in_=d_wih[:])
            nc.sync.dma_start(out=whh_sb[:], in_=d_whh[:])
            nc.sync.dma_start(out=xbias_sb[:], in_=d_xbias[:])
            nc.sync.dma_start(out=wout_sb[:], in_=d_wout[:])
            nc.sync.dma_start(out=bout_sb[:], in_=d_bout[:])
            nc.sync.dma_start(out=transT_sb[:], in_=d_transT[:])
            nc.sync.dma_start(out=transTb_sb[:], in_=d_transTb[:])
            nc.sync.dma_start(out=tstop_sb[:], in_=d_tstop[:])
            nc.sync.dma_start(out=ohprev_sb[:], in_=d_ohprev[:])
            nc.sync.dma_start(out=ohcur_sb[:], in_=d_ohcur[:])
            nc.sync.dma_start(out=ohcur_tb_sb[:], in_=d_ohcur_tb[:])
            nc.sync.dma_start(out=ohlast_sb[:], in_=d_ohlast[:])
            nc.sync.dma_start(out=idxf_sb[:], in_=d_idxf[:])
            nc.sync.dma_start(out=hsT[0][:, :, 0, :], in_=d_h0[:, 0, :, :])
            nc.sync.dma_start(out=hsT[1][:, :, L, :], in_=d_h0[:, 1, :, :])
            for d in (0, 1):
                nc.sync.dma_start(out=cst[d][0][:], in_=d_c0[:, d, :, :])
                nc.vector.memset(cst[d][1][:], 0.0)
            nc.vector.memset(zro[:], 0.0)
            make_identity(nc, ident128[:])
            nc.vector.memset(ones1[:], 1.0)
            nc.vector.memset(ones20c[:], 1.0)
            nc.vector.memset(ones20f[:], 1.0)

            # P+ = exp(transT) in bf16;  wstop = exp(T[STOP,:]) in bf16
            nc.scalar.activation(pplus[:], transT_sb[:], AF.Exp)
            nc.scalar.activation(wstop[:], tstop_sb[:], AF.Exp)

            # A0: time-chain 0 starts at onehot(START); time-chain 1 warms up
            # from uniform over steps NH-CSPEC_W..NH-1, junction-renormalized.
            nc.vector.memset(avec[0][:], 0.0)
            nc.sync.dma_start(out=avec[0][:, 0, :], in_=d_a0[:])
            nc.vector.memset(avec[1][:], 1.0)

            # ---- gold transition score (inputs only; runs during P1) ----
            # pu = trans @ ohprev ; gsum = sum_t (pu * ohcur)
            ps_gold = tc.tile_pool(name="ps_gold", bufs=1, space="PSUM")
            psG = ps_gold.__enter__()
            pu = psG.tile([C, BC * L], F32, tag="pu")
            for n in range(4):
                nc.tensor.matmul(
                    pu[:, n * 512 : (n + 1) * 512],
                    transTb_sb[:],
                    ohprev_sb[:].rearrange("p b t -> p (b t)")[
                        :, n * 512 : (n + 1) * 512
                    ],
                    start=True,
                    stop=True,
                )
            prod = pp.tile([C, BC, L], F32, tag="prod")
            nc.vector.scalar_tensor_tensor(
                out=prod[:].rearrange("p b t -> p (b t)"), in0=pu[:], scalar=0.0,
                in1=ohcur_sb[:].rearrange("p b t -> p (b t)"), op0=ALU.add, op1=ALU.mult,
            )
            nc.vector.tensor_reduce(out=gsum[:], in_=prod[:], axis=AX.X, op=ALU.add)
            # cnt[c,b] = #timesteps with tag c (for the bout term of emissions)
            nc.vector.tensor_reduce(out=cnt_sb[:], in_=ohcur_sb[:], axis=AX.X, op=ALU.add)
            # cnt *= bout (per-tag emission bias counts); gsum += cnt
            nc.gpsimd.tensor_tensor(
                out=cnt_sb[:], in0=cnt_sb[:],
                in1=bout_sb[:].to_broadcast([C, BC]), op=ALU.mult,
            )
            nc.gpsimd.tensor_add(gsum[:], gsum[:], cnt_sb[:])
            ps_gold.__exit__(None, None, None)

            # ---- P1: gather + transpose + input projection ----
            ps_p1 = tc.tile_pool(name="ps_p1", bufs=2, space="PSUM")
            psA = ps_p1.__enter__()
            for ti in range(NT):
                gx = wp.tile([128, E], BF16, tag="gx")
                nc.gpsimd.indirect_dma_start(
                    out=gx[:],
                    out_offset=None,
                    in_=d_embed[:],
                    in_offset=IndirectOffsetOnAxis(ap=idxf_sb[:, ti : ti + 1], axis=0),
                )
                for k in (0, 1):
                    pt = psA.tile([128, 128], BF16, tag="ptr")
                    nc.tensor.transpose(
                        pt[:], gx[:, k * 128 : (k + 1) * 128], ident128[:]
                    )
                    nc.vector.tensor_copy(xTf[:, k, ti, :], pt[:])
            for d in (0, 1):
                for c in range(CH):
                    for h in (0, 1):
                        pj = psA.tile([128, 1024], F32, tag="pj", bufs=3)
                        for k in (0, 1):
                            for nb in (0, 1):
                                nc.tensor.matmul(
                                    pj[:, nb * 512 : (nb + 1) * 512],
                                    wih_sb[:, d, k, c, :],
                                    xTf[:, k, h * 8 + nb * 4 : h * 8 + (nb + 1) * 4, :],
                                    start=(k == 0),
                                    stop=(k == 1),
                                )
                        # xp^T[t, c, b] = pj + bias (split across ACT and DVE)
                        if (c + h) % 2 == 0:
                            nc.scalar.activation(
                                out=xpT[d][:, h * 128 : (h + 1) * 128, c, :],
                                in_=pj[:].rearrange("p (tt b) -> p tt b", b=BC),
                                func=AF.Identity,
                                bias=xbias_sb[:, d, c : c + 1],
                            )
                        else:
                            nc.vector.tensor_scalar(
                                out=xpT[d][:, h * 128 : (h + 1) * 128, c, :],
                                in0=pj[:].rearrange("p (tt b) -> p tt b", b=BC),
                                scalar1=xbias_sb[:, d, c : c + 1],
                                scalar2=None,
                                op0=ALU.add,
                            )
            ps_p1.__exit__(None, None, None)

            # ---- P2: LSTM recurrence, 4 speculative chains ----
            # (dir, half): half 0 covers steps 0..NH-1 exactly; half 1 warms up
            # from zero state over steps NH-SPEC_W..NH-1 (scratch ping-pong),
            # then runs steps NH..L-1 writing the real hsT slots.
            ps_p2 = tc.tile_pool(name="ps_p2", bufs=6, space="PSUM")
            psB = ps_p2.__enter__()

            def chain_slot(d, hf, j):
                # -> (read_view, write_view, xp_time_index) or None
                if hf == 0:
                    if j >= NH:
                        return None
                    s = j
                    if d == 0:
                        rd, wr = hsT[0][:, :, s, :], hsT[0][:, :, s + 1, :]
                    else:
                        rd, wr = hsT[1][:, :, L - s, :], hsT[1][:, :, L - 1 - s, :]
                else:
                    if j >= NH + SPEC_W:
                        return None
                    if j < SPEC_W:
                        s = NH - SPEC_W + j
                        rd = zro[:] if j == 0 else hwu[d][:, :, (j + 1) % 2, :]
                        wr = hwu[d][:, :, j % 2, :]
                    else:
                        s = NH + (j - SPEC_W)
                        if j == SPEC_W:
                            rd = hwu[d][:, :, (SPEC_W - 1) % 2, :]
                        else:
                            rd = (hsT[0][:, :, s, :] if d == 0
                                  else hsT[1][:, :, L - s, :])
                        wr = (hsT[0][:, :, s + 1, :] if d == 0
                              else hsT[1][:, :, L - 1 - s, :])
                xi = s if d == 0 else L - 1 - s
                return rd, wr, xi

            for j in range(NH + SPEC_W):
                for d, hf in ((0, 0), (1, 0), (0, 1), (1, 1)):
                    cs = chain_slot(d, hf, j)
                    if cs is None:
                        continue
                    rd, wr, xi = cs
                    cd = cst[d][hf]
                    pg = psB.tile([128, CH, BC], F32, tag="pg")
                    nc.tensor.matmul(
                        pg[:].rearrange("p c b -> p (c b)"),
                        ident128[:],
                        xpT[d][:, xi, :, :].rearrange("p c b -> p (c b)"),
                        start=True,
                        stop=False,
                        skip_group_check=True,
                    )
                    for c in range(CH):
                        for k in (0, 1):
                            nc.tensor.matmul(
                                pg[:, c, :],
                                whh_sb[:, d, k, c, :],
                                rd[:, k, :],
                                start=False,
                                stop=(c == CH - 1 and k == 1),
                                skip_group_check=True,
                            )
                    th = lp.tile([128, CH, BC], F32, tag="th")
                    nc.scalar.activation(th[:], pg[:], AF.Tanh)
                    v = lp.tile([128, 2, BC], F32, tag="v")
                    nc.vector.scalar_tensor_tensor(
                        out=v[:], in0=th[:, 2:4, :], scalar=1.0, in1=cd[:],
                        op0=ALU.add, op1=ALU.mult,
                    )
                    u = lp.tile([128, 2, BC], F32, tag="u")
                    nc.vector.scalar_tensor_tensor(
                        out=u[:], in0=th[:, 0:2, :], scalar=1.0, in1=th[:, 4:6, :],
                        op0=ALU.add, op1=ALU.mult,
                    )
                    nc.vector.scalar_tensor_tensor(
                        out=cd[:], in0=v[:], scalar=0.5, in1=u[:],
                        op0=ALU.mult, op1=ALU.add,
                    )
                    tcc = lp.tile([128, 2, BC], F32, tag="tcc")
                    nc.scalar.activation(tcc[:], cd[:], AF.Tanh, scale=0.5)
                    nc.vector.scalar_tensor_tensor(
                        out=wr, in0=th[:, 6:8, :], scalar=1.0,
                        in1=tcc[:], op0=ALU.add, op1=ALU.mult,
                    )
            ps_p2.__exit__(None, None, None)

            # ---- P3: emissions E = exp(sum_d Wout_d @ H_d + bout) ----
            ps_p3 = tc.tile_pool(name="ps_p3", bufs=1, space="PSUM")
            psC = ps_p3.__enter__()
            pf = psC.tile([C, L * BC], F32, tag="pf")
            for d in (0, 1):
                for k in (0, 1):
                    for n in range(4):
                        base = 1 + n * 64 if d == 0 else n * 64
                        nc.tensor.matmul(
                            pf[:, n * 512 : (n + 1) * 512],
                            wout_sb[:, d, k, :],
                            hsT[d][:, k, base : base + 64, :],
                            start=(d == 0 and k == 0),
                            stop=(d == 1 and k == 1),
                        )
            nc.scalar.activation(
                out=eT[:].rearrange("p t b -> p (t b)"),
                in_=pf[:],
                func=AF.Exp,
                bias=bout_sb[:, 0:1],
            )
            # gold emissions: sum_t pf[gold tag] (bout term already via cnt)
            prod2 = pp.tile([C, L, BC], F32, tag="prod2")
            nc.vector.scalar_tensor_tensor(
                out=prod2[:].rearrange("p t b -> p (t b)"), in0=pf[:], scalar=0.0,
                in1=ohcur_tb_sb[:].rearrange("p t b -> p (t b)"), op0=ALU.add, op1=ALU.mult,
            )
            gsum2 = pp.tile([C, BC], F32, tag="gsum2")
            nc.vector.tensor_reduce(
                out=gsum2[:],
                in_=prod2[:].rearrange("p t b -> p b t"),
                axis=AX.X, op=ALU.add,
            )
            nc.gpsimd.tensor_add(gsum[:], gsum[:], gsum2[:])
            # + T[STOP, tag_last]
            stopterm = pp.tile([C, BC], F32, tag="stopterm")
            nc.gpsimd.tensor_tensor(
                out=stopterm[:], in0=ohlast_sb[:],
                in1=tstop_sb[:].to_broadcast([C, BC]), op=ALU.mult,
            )
            nc.gpsimd.tensor_add(gsum[:], gsum[:], stopterm[:])
            ps_p3.__exit__(None, None, None)

            # ---- P5: CRF forward scan, two staggered 4-row chains ----
            ps_p5 = tc.tile_pool(name="ps_p5", bufs=2, space="PSUM")
            psD = ps_p5.__enter__()
            # gold reduce over tags (PE while idle-ish): gold = ones20c^T @ gsum
            pgold = psD.tile([1, BC], F32, tag="pgold", bufs=1)
            nc.tensor.matmul(pgold[:], ones20f[:], gsum[:], start=True, stop=True)
            nc.vector.tensor_copy(gold_sb[:], pgold[:])

            for j in range(NH + CSPEC_W):
                for x in (0, 1):
                    if x == 0:
                        if j >= NH:
                            continue
                        t = j
                        warm_end = False
                        log_rn = (t % REN == REN - 1)
                    else:
                        if j < CSPEC_W:
                            t = NH - CSPEC_W + j
                            warm_end = (j == CSPEC_W - 1)
                            log_rn = False
                        else:
                            t = NH + (j - CSPEC_W)
                            warm_end = False
                            log_rn = (t % REN == REN - 1)
                    cur = (j + 1) % 2
                    pa = psD.tile([C, BC], F32, tag=f"pa{x}")
                    nc.tensor.matmul(
                        pa[:], pplus[:], avec[x][:, j % 2, :], start=True, stop=True
                    )
                    nc.vector.scalar_tensor_tensor(
                        out=avec[x][:, cur, :], in0=pa[:], scalar=0.0,
                        in1=eT[:, t, :], op0=ALU.add, op1=ALU.mult,
                    )
                    if log_rn or warm_end:
                        ssum = wp.tile([C, BC], F32, tag=f"ssum{x}")
                        nc.gpsimd.partition_all_reduce(
                            ssum[:], avec[x][:, cur, :], channels=C,
                            reduce_op=bass_isa.ReduceOp.add,
                        )
                        srec = wp.tile([C, BC], F32, tag=f"srec{x}")
                        nc.vector.reciprocal(srec[:], ssum[:])
                        if log_rn:
                            nc.vector.tensor_copy(sall[0:1, :, t // REN], srec[0:1, :])
                        nc.vector.scalar_tensor_tensor(
                            out=avec[x][:, cur, :],
                            in0=avec[x][:, cur, :], scalar=0.0,
                            in1=srec[:], op0=ALU.add, op1=ALU.mult,
                        )

            # ---- P6: ship gold/paf/sall to host (lns done in float64 there;
            # the ACT Ln table is invalid for the ~1e-24 srec magnitudes) ----
            paf = psD.tile([1, BC], F32, tag="paf", bufs=1)
            nc.tensor.matmul(
                paf[:], wstop[:], avec[1][:, (NH + CSPEC_W) % 2, :],
                start=True, stop=True,
            )
            paf_sb = wp.tile([1, BC], F32, tag="paf_sb")
            nc.vector.tensor_copy(paf_sb[:], paf[:])
            nc.sync.dma_start(out=d_gold[:], in_=gold_sb[:])
            nc.sync.dma_start(out=d_paf[:], in_=paf_sb[:])
            nc.sync.dma_start(out=d_sall[:], in_=sall[:])
            if DEBUG:
                nc.sync.dma_start(out=d_dbg_eT[:], in_=eT[:])
                nc.sync.dma_start(out=d_dbg_gsum[:], in_=gsum[:])
                nc.sync.dma_start(out=d_dbg_gold[:], in_=gold_sb[:])
                nc.sync.dma_start(out=d_dbg_hf[:], in_=hsT[0][:, :, L, :])
                nc.sync.dma_start(out=d_dbg_hb[:], in_=hsT[1][:, :, 0, :])
                nc.sync.dma_start(out=d_dbg_sall[:], in_=sall[:])
                nc.sync.dma_start(out=d_dbg_xp[:], in_=xpT[0][:, 0:8, :, :])
                for d in (0, 1):
                    nc.sync.dma_start(out=d_dbg_cst[:, d, :, :], in_=cst[d][:])
            ps_p5.__exit__(None, None, None)

    nc.finalize()
    return nc


def _prep_inmaps(inputs):
    bf = ml_dtypes.bfloat16
    sent = np.asarray(inputs["sentences"])
    tags = np.asarray(inputs["tags"])
    embed = np.asarray(inputs["embed"], dtype=np.float32)
    trans = np.asarray(inputs["transitions"], dtype=np.float32)
    h0 = np.asarray(inputs["h0"], dtype=np.float32)
    c0 = np.asarray(inputs["c0"], dtype=np.float32)
    W_out = np.asarray(inputs["W_out"], dtype=np.float32)
    b_out = np.asarray(inputs["b_out"], dtype=np.float32)

    rs = np.full((G, 1), 0.5, np.float32)
    rs[2 * H : 3 * H] = 1.0  # g-gate rows unscaled

    embed_bf = np.ascontiguousarray(embed.astype(bf))

    def chunk_weights(W):  # W [G, K_in] -> [128, 2, CH, 128] = [p, k, c, m]
        Kin = W.shape[1]
        Wr = W.reshape(4, 2, 128, Kin // 128, 128)  # [gate, hh, m, k, p]
        return np.ascontiguousarray(Wr.transpose(4, 3, 0, 1, 2).reshape(128, Kin // 128, CH, 128))

    wih = np.zeros((128, 2, 2, CH, 128), np.float32)
    whh = np.zeros((128, 2, 2, CH, 128), np.float32)
    xbias = np.zeros((128, 2, CH), np.float32)
    for d, (Wih, Whh, b) in enumerate(
        [
            (inputs["Wih_f"], inputs["Whh_f"], inputs["b_f"]),
            (inputs["Wih_b"], inputs["Whh_b"], inputs["b_b"]),
        ]
    ):
        Wih = np.asarray(Wih, np.float32) * rs
        Whh = np.asarray(Whh, np.float32) * rs * 0.5
        bt = np.asarray(b, np.float32) * rs[:, 0]
        wih[:, d] = chunk_weights(Wih)
        whh[:, d] = chunk_weights(Whh)
        xbias[:, d] = bt.reshape(4, 2, 128).transpose(2, 0, 1).reshape(128, CH)
    wih = np.ascontiguousarray(wih.astype(bf))
    whh = np.ascontiguousarray(whh.astype(bf))

    # wout^T [p, d, k, m] = 0.5 * W_out[m, d*256 + k*128 + p]
    wout = np.ascontiguousarray(
        (0.5 * W_out).reshape(C, 2, 2, 128).transpose(3, 1, 2, 0).astype(bf)
    )
    bout = np.ascontiguousarray(b_out[:, None])
    transT = np.ascontiguousarray(trans.T)
    transTb = np.ascontiguousarray(trans.T.astype(bf))
    tstop = np.ascontiguousarray(trans[STOP, :][:, None])

    in_maps = []
    for q in range(NCORES):
        bs = slice(q * BC, (q + 1) * BC)
        sq = sent[bs]  # [BC, L]
        tq = tags[bs]
        idx_f = np.ascontiguousarray(
            sq.T.reshape(NT, TPT, BC).transpose(1, 2, 0).reshape(128, NT).astype(np.int32)
        )
        h0q = np.ascontiguousarray(
            (2.0 * h0[:, bs, :]).reshape(2, BC, 2, 128).transpose(3, 0, 2, 1).astype(bf)
        )
        c0q = np.ascontiguousarray(
            (2.0 * c0[:, bs, :]).reshape(2, BC, 2, 128).transpose(3, 0, 2, 1).astype(np.float32)
        )
        te_prev = np.concatenate(
            [np.full((BC, 1), START, tags.dtype), tq[:, :-1]], axis=1
        )  # prev tag at each t
        ar = np.arange(C)
        ohprev = (ar[:, None, None] == te_prev[None, :, :]).astype(np.float32)
        ohcur = (ar[:, None, None] == tq[None, :, :]).astype(np.float32)
        ohcur_tb = np.ascontiguousarray(ohcur.transpose(0, 2, 1))
        ohlast = (ar[:, None] == tq[None, :, L - 1]).astype(np.float32)
        a0 = ((ar[:, None] == START) * np.ones((1, BC))).astype(bf)
        in_maps.append(
            {
                "embed_bf": embed_bf,
                "idx_f": idx_f,
                "wih": wih,
                "whh": whh,
                "xbias": xbias,
                "h0T": h0q,
                "c0T": c0q,
                "woutT": wout,
                "bout": bout,
                "transT": transT,
                "transTb": transTb,
                "tstop": tstop,
                "ohprev": np.ascontiguousarray(ohprev.astype(bf)),
                "ohcur": np.ascontiguousarray(ohcur),
                "ohcur_tb": ohcur_tb,
                "ohlast": np.ascontiguousarray(ohlast),
                "a0": np.ascontiguousarray(a0),
            }
        )
    return in_maps


def get_module():
    if "nc" not in _CACHE:
        _CACHE["nc"] = _build_module()
    return _CACHE["nc"]


def _finalize(outs):
    """Host-side: partial = sum_b [ln(paf_b) - sum_r ln(srec_br) - gold_b]."""
    paf = np.asarray(outs["paf_out"], np.float64)[0]
    sall = np.asarray(outs["sall_out"], np.float64)[0]
    gold = np.asarray(outs["gold_out"], np.float64)[0]
    F = np.log(paf) - np.log(sall).sum(axis=1)
    return float((F - gold).sum())


def kernel(**inputs):
    nc = get_module()
    in_maps = _prep_inmaps(inputs)
    res = run_bass_kernel_spmd(nc, in_maps, core_ids=list(range(NCORES)))
    total = sum(_finalize(r) for r in res.results)
    return np.float32(total / B)


# revision 9
# speedup vs baseline: 2.1818x; 1.0301x over previous
"""BiLSTM-CRF loss kernel for 8 Trainium2 NeuronCores (v2).

Sharding: data-parallel over batch (64 -> 8 cores x 8 rows). Each core runs
both LSTM directions for its batch shard, computes CRF emissions, runs the
CRF forward pass in the exp domain, the gold-path score, and writes the
partial sum of (forward - gold) over its 8 rows. Host sums partials / 64.

v2 changes vs v1:
  - Single gather/transpose set: backward direction consumes forward-time
    xpT at reversed indices and writes hsT at reversed slots, so hs_b ends
    up in forward-time order for the emission matmuls.
  - Input projection xp is injected into the LSTM gate PSUM via an
    identity-stationary matmul, eliminating the per-step DVE add; tanh
    reads PSUM directly on the Activation engine.
  - The v gate product runs on GpSimd, balancing DVE.
  - P1 bias-adds and P3's exp(feats) run on the Activation engine with
    per-partition bias (bout folded into the exp).
  - No max-shift in the CRF: E = exp(W h + b) directly; periodic renorm
    keeps the exp-domain scan in f32 range.
  - CRF scan in bf16 (single HW matmul per step instead of an fp32 pair),
    split into two staggered 4-row chains with renorm every 16 steps.
"""

import sys

sys.path.insert(0, "/opt/trn_rl_repo")

import numpy as np
import ml_dtypes

import concourse.bass as bass
from concourse import bacc
import concourse.tile as tile
from concourse import mybir
from concourse import bass_isa
from concourse.bass import IndirectOffsetOnAxis
from concourse.bass_utils import run_bass_kernel_spmd
from concourse.masks import make_identity

F32 = mybir.dt.float32
BF16 = mybir.dt.bfloat16
I32 = mybir.dt.int32
ALU = mybir.AluOpType
AF = mybir.ActivationFunctionType
AX = mybir.AxisListType

B, L, E, H, C = 64, 256, 256, 256, 20
G = 4 * H
NCORES = 8
BC = B // NCORES            # batch rows per core
CH = 8                      # gate-hidden chunks of 128 (c = gate*2 + half)
NT = (L * BC) // 128        # token tiles = 16
TPT = 128 // BC             # timesteps per token tile = 16
REN = 16                    # CRF renorm interval (per chain)
NREN = L // REN             # 16
HBC = BC // 2               # rows per CRF chain = 4
START, STOP = 18, 19

_CACHE = {}
DEBUG = False
SPEC_W = 12    # LSTM speculative warmup steps
CSPEC_W = 12   # CRF speculative warmup steps
NH = L // 2     # 128


def _build_module():
    nc = bacc.Bacc(None, target_bir_lowering=False, debug=False)

    # ---- DRAM I/O ----
    d_embed = nc.dram_tensor("embed_bf", [50000, E], BF16, kind="ExternalInput")
    d_idxf = nc.dram_tensor("idx_f", [128, NT], I32, kind="ExternalInput")
    d_wih = nc.dram_tensor("wih", [128, 2, 2, CH, 128], BF16, kind="ExternalInput")
    d_whh = nc.dram_tensor("whh", [128, 2, 2, CH, 128], BF16, kind="ExternalInput")
    d_xbias = nc.dram_tensor("xbias", [128, 2, CH], F32, kind="ExternalInput")
    d_h0 = nc.dram_tensor("h0T", [128, 2, 2, BC], BF16, kind="ExternalInput")
    d_c0 = nc.dram_tensor("c0T", [128, 2, 2, BC], F32, kind="ExternalInput")
    d_wout = nc.dram_tensor("woutT", [128, 2, 2, C], BF16, kind="ExternalInput")
    d_bout = nc.dram_tensor("bout", [C, 1], F32, kind="ExternalInput")
    d_transT = nc.dram_tensor("transT", [C, C], F32, kind="ExternalInput")
    d_transTb = nc.dram_tensor("transTb", [C, C], BF16, kind="ExternalInput")
    d_tstop = nc.dram_tensor("tstop", [C, 1], F32, kind="ExternalInput")
    d_ohprev = nc.dram_tensor("ohprev", [C, BC, L], BF16, kind="ExternalInput")
    d_ohcur = nc.dram_tensor("ohcur", [C, BC, L], F32, kind="ExternalInput")
    d_ohcur_tb = nc.dram_tensor("ohcur_tb", [C, L, BC], F32, kind="ExternalInput")
    d_ohlast = nc.dram_tensor("ohlast", [C, BC], F32, kind="ExternalInput")
    d_a0 = nc.dram_tensor("a0", [C, BC], BF16, kind="ExternalInput")
    d_gold = nc.dram_tensor("gold_out", [1, BC], F32, kind="ExternalOutput")
    d_paf = nc.dram_tensor("paf_out", [1, BC], F32, kind="ExternalOutput")
    d_sall = nc.dram_tensor("sall_out", [1, BC, NREN], F32, kind="ExternalOutput")
    if DEBUG:
        d_dbg_eT = nc.dram_tensor("dbg_eT", [C, L, BC], F32, kind="ExternalOutput")
        d_dbg_gsum = nc.dram_tensor("dbg_gsum", [C, BC], F32, kind="ExternalOutput")
        d_dbg_gold = nc.dram_tensor("dbg_gold", [1, BC], F32, kind="ExternalOutput")
        d_dbg_hf = nc.dram_tensor("dbg_hf", [128, 2, BC], BF16, kind="ExternalOutput")
        d_dbg_hb = nc.dram_tensor("dbg_hb", [128, 2, BC], BF16, kind="ExternalOutput")
        d_dbg_sall = nc.dram_tensor("dbg_sall", [1, BC, NREN], F32, kind="ExternalOutput")
        d_dbg_xp = nc.dram_tensor("dbg_xp", [128, 8, CH, BC], BF16, kind="ExternalOutput")
        d_dbg_cst = nc.dram_tensor("dbg_cst", [128, 2, 2, BC], F32, kind="ExternalOutput")

    with tile.TileContext(nc) as tc:
        with (
            tc.tile_pool(name="persist", bufs=1) as pp,
            tc.tile_pool(name="work", bufs=3) as wp,
            tc.tile_pool(name="lstm", bufs=3) as lp,
        ):
            # ---- persistent SBUF ----
            wih_sb = pp.tile([128, 2, 2, CH, 128], BF16, tag="wih")
            whh_sb = pp.tile([128, 2, 2, CH, 128], BF16, tag="whh")
            xbias_sb = pp.tile([128, 2, CH], F32, tag="xbias")
            wout_sb = pp.tile([128, 2, 2, C], BF16, tag="wout")
            bout_sb = pp.tile([C, 1], F32, tag="bout")
            transT_sb = pp.tile([C, C], F32, tag="transT")
            transTb_sb = pp.tile([C, C], BF16, tag="transTb")
            tstop_sb = pp.tile([C, 1], F32, tag="tstop")
            ohprev_sb = pp.tile([C, BC, L], BF16, tag="ohprev")
            ohcur_sb = pp.tile([C, BC, L], F32, tag="ohcur")
            ohcur_tb_sb = pp.tile([C, L, BC], F32, tag="ohcur_tb")
            ohlast_sb = pp.tile([C, BC], F32, tag="ohlast")
            idxf_sb = pp.tile([128, NT], I32, tag="idxf")
            ident128 = pp.tile([128, 128], BF16, tag="id128")
            xTf = pp.tile([128, 2, NT, 128], BF16, tag="xTf")
            # xp^T: [ghid-part, t, chunk, b]  (both dirs in forward time order)
            xpT = [pp.tile([128, L, CH, BC], BF16, name=f"xpT{d}", tag=f"xpT{d}") for d in (0, 1)]
            # H history: [hid-part, k-half, slot(0..L), b]
            # dir0: init slot 0, step s reads s, writes s+1 (h_f[t] at slot t+1)
            # dir1: init slot L, step s reads L-s, writes L-1-s (h_b[t] at slot t)
            hsT = [pp.tile([128, 2, L + 1, BC], BF16, name=f"hsT{d}", tag=f"hsT{d}") for d in (0, 1)]
            cst = [[pp.tile([128, 2, BC], F32, name=f"cst{d}{hf}", tag=f"cst{d}{hf}")
                    for hf in (0, 1)] for d in (0, 1)]
            hwu = [pp.tile([128, 2, 2, BC], BF16, name=f"hwu{d}", tag=f"hwu{d}") for d in (0, 1)]
            zro = pp.tile([128, 2, BC], BF16, tag="zro")
            eT = pp.tile([C, L, BC], F32, tag="eT")
            pplus = pp.tile([C, C], BF16, tag="pplus")
            wstop = pp.tile([C, 1], BF16, tag="wstop")
            # applied renorm scales (exact f32 bookkeeping): [1, chain*HBC, NREN]
            sall = pp.tile([1, BC, NREN], F32, tag="sall")
            ones1 = pp.tile([1, C], F32, tag="ones1")
            ones20c = pp.tile([C, 1], BF16, tag="ones20c")
            ones20f = pp.tile([C, 1], F32, tag="ones20f")
            avec = [pp.tile([C, 2, BC], BF16, name=f"avec{x}", tag=f"avec{x}") for x in range(4)]
            gsum = pp.tile([C, BC], F32, tag="gsum")
            gold_sb = pp.tile([1, BC], F32, tag="gold")
            cnt_sb = pp.tile([C, BC], F32, tag="cnt")

            # ---- load constants ----
            nc.sync.dma_start(out=wih_sb[:], in_=d_wih[:])
            nc.sync.dma_start(out=whh_sb[:], in_=d_whh[:])
            nc.sync.dma_start(out=xbias_sb[:], in_=d_xbias[:])
            nc.sync.dma_start(out=wout_sb[:], in_=d_wout[:])
            nc.sync.dma_start(out=bout_sb[:], in_=d_bout[:])
            nc.sync.dma_start(out=transT_sb[:], in_=d_transT[:])
            nc.sync.dma_start(out=transTb_sb[:], in_=d_transTb[:])
            nc.sync.dma_start(out=tstop_sb[:], in_=d_tstop[:])
            nc.sync.dma_start(out=ohprev_sb[:], in_=d_ohprev[:])
            nc.sync.dma_start(out=ohcur_sb[:], in_=d_ohcur[:])
            nc.sync.dma_start(out=ohcur_tb_sb[:], in_=d_ohcur_tb[:])
            nc.sync.dma_start(out=ohlast_sb[:], in_=d_ohlast[:])
            nc.sync.dma_start(out=idxf_sb[:], in_=d_idxf[:])
            nc.sync.dma_start(out=hsT[0][:, :, 0, :], in_=d_h0[:, 0, :, :])
            nc.sync.dma_start(out=hsT[1][:, :, L, :], in_=d_h0[:, 1, :, :])
            for d in (0, 1):
                nc.sync.dma_start(out=cst[d][0][:], in_=d_c0[:, d, :, :])
                nc.vector.memset(cst[d][1][:], 0.0)
            nc.vector.memset(zro[:], 0.0)
            make_identity(nc, ident128[:])
            nc.vector.memset(ones1[:], 1.0)
            nc.vector.memset(ones20c[:], 1.0)
            nc.vector.memset(ones20f[:], 1.0)

            # P+ = exp(transT) in bf16;  wstop = exp(T[STOP,:]) in bf16
            nc.scalar.activation(pplus[:], transT_sb[:], AF.Exp)
            nc.scalar.activation(wstop[:], tstop_sb[:], AF.Exp)

            # A0: time-chain 0 starts at onehot(START); chains 1-3 warm up
            # from uniform over CSPEC_W steps, junction-renormalized.
            nc.vector.memset(avec[0][:], 0.0)
            nc.sync.dma_start(out=avec[0][:, 0, :], in_=d_a0[:])
            for x in (1, 2, 3):
                nc.vector.memset(avec[x][:], 1.0)

            # ---- gold transition score (inputs only; runs during P1) ----
            # pu = trans @ ohprev ; gsum = sum_t (pu * ohcur)
            ps_gold = tc.tile_pool(name="ps_gold", bufs=1, space="PSUM")
            psG = ps_gold.__enter__()
            pu = psG.tile([C, BC * L], F32, tag="pu")
            for n in range(4):
                nc.tensor.matmul(
                    pu[:, n * 512 : (n + 1) * 512],
                    transTb_sb[:],
                    ohprev_sb[:].rearrange("p b t -> p (b t)")[
                        :, n * 512 : (n + 1) * 512
                    ],
                    start=True,
                    stop=True,
                )
            prod = pp.tile([C, BC, L], F32, tag="prod")
            nc.vector.scalar_tensor_tensor(
                out=prod[:].rearrange("p b t -> p (b t)"), in0=pu[:], scalar=0.0,
                in1=ohcur_sb[:].rearrange("p b t -> p (b t)"), op0=ALU.add, op1=ALU.mult,
            )
            nc.vector.tensor_reduce(out=gsum[:], in_=prod[:], axis=AX.X, op=ALU.add)
            # cnt[c,b] = #timesteps with tag c (for the bout term of emissions)
            nc.vector.tensor_reduce(out=cnt_sb[:], in_=ohcur_sb[:], axis=AX.X, op=ALU.add)
            # cnt *= bout (per-tag emission bias counts); gsum += cnt
            nc.gpsimd.tensor_tensor(
                out=cnt_sb[:], in0=cnt_sb[:],
                in1=bout_sb[:].to_broadcast([C, BC]), op=ALU.mult,
            )
            nc.gpsimd.tensor_add(gsum[:], gsum[:], cnt_sb[:])
            ps_gold.__exit__(None, None, None)

            # ---- P1: gather + transpose + input projection ----
            # Projection groups for (d0,h0) are emitted right after tile 0-7
            # transposes so they execute while gathers 8-15 are still running.
            ps_p1 = tc.tile_pool(name="ps_p1", bufs=2, space="PSUM")
            psA = ps_p1.__enter__()

            def gather_transpose(ti):
                gx = wp.tile([128, E], BF16, tag="gx")
                nc.gpsimd.indirect_dma_start(
                    out=gx[:],
                    out_offset=None,
                    in_=d_embed[:],
                    in_offset=IndirectOffsetOnAxis(ap=idxf_sb[:, ti : ti + 1], axis=0),
                )
                for k in (0, 1):
                    pt = psA.tile([128, 128], BF16, tag="ptr")
                    nc.tensor.transpose(
                        pt[:], gx[:, k * 128 : (k + 1) * 128], ident128[:]
                    )
                    nc.vector.tensor_copy(xTf[:, k, ti, :], pt[:])

            def proj(d, h):
                for c in range(CH):
                    pj = psA.tile([128, 1024], F32, tag="pj", bufs=3)
                    for k in (0, 1):
                        for nb in (0, 1):
                            nc.tensor.matmul(
                                pj[:, nb * 512 : (nb + 1) * 512],
                                wih_sb[:, d, k, c, :],
                                xTf[:, k, h * 8 + nb * 4 : h * 8 + (nb + 1) * 4, :],
                                start=(k == 0),
                                stop=(k == 1),
                            )
                    # xp^T[t, c, b] = pj + bias (split across ACT and DVE)
                    if (c + h) % 2 == 0:
                        nc.scalar.activation(
                            out=xpT[d][:, h * 128 : (h + 1) * 128, c, :],
                            in_=pj[:].rearrange("p (tt b) -> p tt b", b=BC),
                            func=AF.Identity,
                            bias=xbias_sb[:, d, c : c + 1],
                        )
                    else:
                        nc.vector.tensor_scalar(
                            out=xpT[d][:, h * 128 : (h + 1) * 128, c, :],
                            in0=pj[:].rearrange("p (tt b) -> p tt b", b=BC),
                            scalar1=xbias_sb[:, d, c : c + 1],
                            scalar2=None,
                            op0=ALU.add,
                        )

            for ti in range(8):
                gather_transpose(ti)
            proj(0, 0)
            for ti in range(8, NT):
                gather_transpose(ti)
            proj(1, 1)
            proj(1, 0)
            proj(0, 1)
            ps_p1.__exit__(None, None, None)

            # ---- P2: LSTM recurrence, 4 speculative chains ----
            # (dir, half): half 0 covers steps 0..NH-1 exactly; half 1 warms up
            # from zero state over steps NH-SPEC_W..NH-1 (scratch ping-pong),
            # then runs steps NH..L-1 writing the real hsT slots.
            ps_p2 = tc.tile_pool(name="ps_p2", bufs=6, space="PSUM")
            psB = ps_p2.__enter__()

            def chain_slot(d, hf, j):
                # -> (read_view, write_view, xp_time_index) or None
                if hf == 0:
                    if j >= NH:
                        return None
                    s = j
                    if d == 0:
                        rd, wr = hsT[0][:, :, s, :], hsT[0][:, :, s + 1, :]
                    else:
                        rd, wr = hsT[1][:, :, L - s, :], hsT[1][:, :, L - 1 - s, :]
                else:
                    if j >= NH + SPEC_W:
                        return None
                    if j < SPEC_W:
                        s = NH - SPEC_W + j
                        rd = zro[:] if j == 0 else hwu[d][:, :, (j + 1) % 2, :]
                        wr = hwu[d][:, :, j % 2, :]
                    else:
                        s = NH + (j - SPEC_W)
                        if j == SPEC_W:
                            rd = hwu[d][:, :, (SPEC_W - 1) % 2, :]
                        else:
                            rd = (hsT[0][:, :, s, :] if d == 0
                                  else hsT[1][:, :, L - s, :])
                        wr = (hsT[0][:, :, s + 1, :] if d == 0
                              else hsT[1][:, :, L - 1 - s, :])
                xi = s if d == 0 else L - 1 - s
                return rd, wr, xi

            for j in range(NH + SPEC_W):
                for d, hf in ((0, 0), (1, 0), (0, 1), (1, 1)):
                    cs = chain_slot(d, hf, j)
                    if cs is None:
                        continue
                    rd, wr, xi = cs
                    cd = cst[d][hf]
                    pg = psB.tile([128, CH, BC], F32, tag="pg")
                    nc.tensor.matmul(
                        pg[:].rearrange("p c b -> p (c b)"),
                        ident128[:],
                        xpT[d][:, xi, :, :].rearrange("p c b -> p (c b)"),
                        start=True,
                        stop=False,
                        skip_group_check=True,
                    )
                    for c in range(CH):
                        for k in (0, 1):
                            nc.tensor.matmul(
                                pg[:, c, :],
                                whh_sb[:, d, k, c, :],
                                rd[:, k, :],
                                start=False,
                                stop=(c == CH - 1 and k == 1),
                                skip_group_check=True,
                            )
                    th = lp.tile([128, CH, BC], F32, tag="th")
                    nc.scalar.activation(th[:], pg[:], AF.Tanh)
                    v = lp.tile([128, 2, BC], F32, tag="v")
                    nc.vector.scalar_tensor_tensor(
                        out=v[:], in0=th[:, 2:4, :], scalar=1.0, in1=cd[:],
                        op0=ALU.add, op1=ALU.mult,
                    )
                    u = lp.tile([128, 2, BC], F32, tag="u")
                    nc.vector.scalar_tensor_tensor(
                        out=u[:], in0=th[:, 0:2, :], scalar=1.0, in1=th[:, 4:6, :],
                        op0=ALU.add, op1=ALU.mult,
                    )
                    nc.vector.scalar_tensor_tensor(
                        out=cd[:], in0=v[:], scalar=0.5, in1=u[:],
                        op0=ALU.mult, op1=ALU.add,
                    )
                    tcc = lp.tile([128, 2, BC], F32, tag="tcc")
                    nc.scalar.activation(tcc[:], cd[:], AF.Tanh, scale=0.5)
                    nc.vector.scalar_tensor_tensor(
                        out=wr, in0=th[:, 6:8, :], scalar=1.0,
                        in1=tcc[:], op0=ALU.add, op1=ALU.mult,
                    )
            ps_p2.__exit__(None, None, None)

            # ---- P3: emissions E = exp(sum_d Wout_d @ H_d + bout) ----
            ps_p3 = tc.tile_pool(name="ps_p3", bufs=1, space="PSUM")
            psC = ps_p3.__enter__()
            pf = psC.tile([C, L * BC], F32, tag="pf")
            for d in (0, 1):
                for k in (0, 1):
                    for n in range(4):
                        base = 1 + n * 64 if d == 0 else n * 64
                        nc.tensor.matmul(
                            pf[:, n * 512 : (n + 1) * 512],
                            wout_sb[:, d, k, :],
                            hsT[d][:, k, base : base + 64, :],
                            start=(d == 0 and k == 0),
                            stop=(d == 1 and k == 1),
                        )
            nc.scalar.activation(
                out=eT[:].rearrange("p t b -> p (t b)"),
                in_=pf[:],
                func=AF.Exp,
                bias=bout_sb[:, 0:1],
            )
            # gold emissions: sum_t pf[gold tag] (bout term already via cnt)
            prod2 = pp.tile([C, L, BC], F32, tag="prod2")
            nc.vector.scalar_tensor_tensor(
                out=prod2[:].rearrange("p t b -> p (t b)"), in0=pf[:], scalar=0.0,
                in1=ohcur_tb_sb[:].rearrange("p t b -> p (t b)"), op0=ALU.add, op1=ALU.mult,
            )
            gsum2 = pp.tile([C, BC], F32, tag="gsum2")
            nc.vector.tensor_reduce(
                out=gsum2[:],
                in_=prod2[:].rearrange("p t b -> p b t"),
                axis=AX.X, op=ALU.add,
            )
            nc.gpsimd.tensor_add(gsum[:], gsum[:], gsum2[:])
            # + T[STOP, tag_last]
            stopterm = pp.tile([C, BC], F32, tag="stopterm")
            nc.gpsimd.tensor_tensor(
                out=stopterm[:], in0=ohlast_sb[:],
                in1=tstop_sb[:].to_broadcast([C, BC]), op=ALU.mult,
            )
            nc.gpsimd.tensor_add(gsum[:], gsum[:], stopterm[:])
            ps_p3.__exit__(None, None, None)

            # ---- P5: CRF forward scan, two staggered 4-row chains ----
            ps_p5 = tc.tile_pool(name="ps_p5", bufs=2, space="PSUM")
            psD = ps_p5.__enter__()
            # gold reduce over tags (PE while idle-ish): gold = ones20c^T @ gsum
            pgold = psD.tile([1, BC], F32, tag="pgold", bufs=1)
            nc.tensor.matmul(pgold[:], ones20f[:], gsum[:], start=True, stop=True)
            nc.vector.tensor_copy(gold_sb[:], pgold[:])

            NQ = L // 4
            for j in range(NQ + CSPEC_W):
                for x in range(4):
                    if x == 0:
                        if j >= NQ:
                            continue
                        t = j
                        warm_end = False
                        log_rn = (t % REN == REN - 1)
                    else:
                        if j < CSPEC_W:
                            t = x * NQ - CSPEC_W + j
                            warm_end = (j == CSPEC_W - 1)
                            log_rn = False
                        else:
                            t = x * NQ + (j - CSPEC_W)
                            warm_end = False
                            log_rn = (t % REN == REN - 1)
                    cur = (j + 1) % 2
                    pa = psD.tile([C, BC], F32, tag="pa", bufs=6)
                    nc.tensor.matmul(
                        pa[:], pplus[:], avec[x][:, j % 2, :], start=True, stop=True
                    )
                    nc.vector.scalar_tensor_tensor(
                        out=avec[x][:, cur, :], in0=pa[:], scalar=0.0,
                        in1=eT[:, t, :], op0=ALU.add, op1=ALU.mult,
                    )
                    if log_rn or warm_end:
                        ssum = wp.tile([C, BC], F32, tag=f"ssum{x}")
                        nc.gpsimd.partition_all_reduce(
                            ssum[:], avec[x][:, cur, :], channels=C,
                            reduce_op=bass_isa.ReduceOp.add,
                        )
                        srec = wp.tile([C, BC], F32, tag=f"srec{x}")
                        nc.vector.reciprocal(srec[:], ssum[:])
                        if log_rn:
                            nc.vector.tensor_copy(sall[0:1, :, t // REN], srec[0:1, :])
                        nc.vector.scalar_tensor_tensor(
                            out=avec[x][:, cur, :],
                            in0=avec[x][:, cur, :], scalar=0.0,
                            in1=srec[:], op0=ALU.add, op1=ALU.mult,
                        )

            # ---- P6: ship gold/paf/sall to host (lns done in float64 there;
            # the ACT Ln table is invalid for the ~1e-24 srec magnitudes) ----
            paf = psD.tile([1, BC], F32, tag="paf", bufs=1)
            nc.tensor.matmul(
                paf[:], wstop[:], avec[3][:, (NQ + CSPEC_W) % 2, :],
                start=True, stop=True,
            )
            paf_sb = wp.tile([1, BC], F32, tag="paf_sb")
            nc.vector.tensor_copy(paf_sb[:], paf[:])
            nc.sync.dma_start(out=d_gold[:], in_=gold_sb[:])
            nc.sync.dma_start(out=d_paf[:], in_=paf_sb[:])
            nc.sync.dma_start(out=d_sall[:], in_=sall[:])
            if DEBUG:
                nc.sync.dma_start(out=d_dbg_eT[:], in_=eT[:])
                nc.sync.dma_start(out=d_dbg_gsum[:], in_=gsum[:])
                nc.sync.dma_start(out=d_dbg_gold[:], in_=gold_sb[:])
                nc.sync.dma_start(out=d_dbg_hf[:], in_=hsT[0][:, :, L, :])
                nc.sync.dma_start(out=d_dbg_hb[:], in_=hsT[1][:, :, 0, :])
                nc.sync.dma_start(out=d_dbg_sall[:], in_=sall[:])
                nc.sync.dma_start(out=d_dbg_xp[:], in_=xpT[0][:, 0:8, :, :])
                for d in (0, 1):
                    nc.sync.dma_start(out=d_dbg_cst[:, d, :, :], in_=cst[d][:])
            ps_p5.__exit__(None, None, None)

    nc.finalize()
    return nc


def _prep_inmaps(inputs):
    bf = ml_dtypes.bfloat16
    sent = np.asarray(inputs["sentences"])
    tags = np.asarray(inputs["tags"])
    embed = np.asarray(inputs["embed"], dtype=np.float32)
    trans = np.asarray(inputs["transitions"], dtype=np.float32)
    h0 = np.asarray(inputs["h0"], dtype=np.float32)
    c0 = np.asarray(inputs["c0"], dtype=np.float32)
    W_out = np.asarray(inputs["W_out"], dtype=np.float32)
    b_out = np.asarray(inputs["b_out"], dtype=np.float32)

    rs = np.full((G, 1), 0.5, np.float32)
    rs[2 * H : 3 * H] = 1.0  # g-gate rows unscaled

    embed_bf = np.ascontiguousarray(embed.astype(bf))

    def chunk_weights(W):  # W [G, K_in] -> [128, 2, CH, 128] = [p, k, c, m]
        Kin = W.shape[1]
        Wr = W.reshape(4, 2, 128, Kin // 128, 128)  # [gate, hh, m, k, p]
        return np.ascontiguousarray(Wr.transpose(4, 3, 0, 1, 2).reshape(128, Kin // 128, CH, 128))

    wih = np.zeros((128, 2, 2, CH, 128), np.float32)
    whh = np.zeros((128, 2, 2, CH, 128), np.float32)
    xbias = np.zeros((128, 2, CH), np.float32)
    for d, (Wih, Whh, b) in enumerate(
        [
            (inputs["Wih_f"], inputs["Whh_f"], inputs["b_f"]),
            (inputs["Wih_b"], inputs["Whh_b"], inputs["b_b"]),
        ]
    ):
        Wih = np.asarray(Wih, np.float32) * rs
        Whh = np.asarray(Whh, np.float32) * rs * 0.5
        bt = np.asarray(b, np.float32) * rs[:, 0]
        wih[:, d] = chunk_weights(Wih)
        whh[:, d] = chunk_weights(Whh)
        xbias[:, d] = bt.reshape(4, 2, 128).transpose(2, 0, 1).reshape(128, CH)
    wih = np.ascontiguousarray(wih.astype(bf))
    whh = np.ascontiguousarray(whh.astype(bf))

    # wout^T [p, d, k, m] = 0.5 * W_out[m, d*256 + k*128 + p]
    wout = np.ascontiguousarray(
        (0.5 * W_out).reshape(C, 2, 2, 128).transpose(3, 1, 2, 0).astype(bf)
    )
    bout = np.ascontiguousarray(b_out[:, None])
    transT = np.ascontiguousarray(trans.T)
    transTb = np.ascontiguousarray(trans.T.astype(bf))
    tstop = np.ascontiguousarray(trans[STOP, :][:, None])

    in_maps = []
    for q in range(NCORES):
        bs = slice(q * BC, (q + 1) * BC)
        sq = sent[bs]  # [BC, L]
        tq = tags[bs]
        idx_f = np.ascontiguousarray(
            sq.T.reshape(NT, TPT, BC).transpose(1, 2, 0).reshape(128, NT).astype(np.int32)
        )
        h0q = np.ascontiguousarray(
            (2.0 * h0[:, bs, :]).reshape(2, BC, 2, 128).transpose(3, 0, 2, 1).astype(bf)
        )
        c0q = np.ascontiguousarray(
            (2.0 * c0[:, bs, :]).reshape(2, BC, 2, 128).transpose(3, 0, 2, 1).astype(np.float32)
        )
        te_prev = np.concatenate(
            [np.full((BC, 1), START, tags.dtype), tq[:, :-1]], axis=1
        )  # prev tag at each t
        ar = np.arange(C)
        ohprev = (ar[:, None, None] == te_prev[None, :, :]).astype(np.float32)
        ohcur = (ar[:, None, None] == tq[None, :, :]).astype(np.float32)
        ohcur_tb = np.ascontiguousarray(ohcur.transpose(0, 2, 1))
        ohlast = (ar[:, None] == tq[None, :, L - 1]).astype(np.float32)
        a0 = ((ar[:, None] == START) * np.ones((1, BC))).astype(bf)
        in_maps.append(
            {
                "embed_bf": embed_bf,
                "idx_f": idx_f,
                "wih": wih,
                "whh": whh,
                "xbias": xbias,
                "h0T": h0q,
                "c0T": c0q,
                "woutT": wout,
                "bout": bout,
                "transT": transT,
                "transTb": transTb,
                "tstop": tstop,
                "ohprev": np.ascontiguousarray(ohprev.astype(bf)),
                "ohcur": np.ascontiguousarray(ohcur),
                "ohcur_tb": ohcur_tb,
                "ohlast": np.ascontiguousarray(ohlast),
                "a0": np.ascontiguousarray(a0),
            }
        )
    return in_maps


def get_module():
    if "nc" not in _CACHE:
        _CACHE["nc"] = _build_module()
    return _CACHE["nc"]


def _finalize(outs):
    """Host-side: partial = sum_b [ln(paf_b) - sum_r ln(srec_br) - gold_b]."""
    paf = np.asarray(outs["paf_out"], np.float64)[0]
    sall = np.asarray(outs["sall_out"], np.float64)[0]
    gold = np.asarray(outs["gold_out"], np.float64)[0]
    F = np.log(paf) - np.log(sall).sum(axis=1)
    return float((F - gold).sum())


def kernel(**inputs):
    nc = get_module()
    in_maps = _prep_inmaps(inputs)
    res = run_bass_kernel_spmd(nc, in_maps, core_ids=list(range(NCORES)))
    total = sum(_finalize(r) for r in res.results)
    return np.float32(total / B)


# revision 10
# speedup vs baseline: 2.2161x; 1.0158x over previous
"""BiLSTM-CRF loss kernel for 8 Trainium2 NeuronCores (v2).

Sharding: data-parallel over batch (64 -> 8 cores x 8 rows). Each core runs
both LSTM directions for its batch shard, computes CRF emissions, runs the
CRF forward pass in the exp domain, the gold-path score, and writes the
partial sum of (forward - gold) over its 8 rows. Host sums partials / 64.

v2 changes vs v1:
  - Single gather/transpose set: backward direction consumes forward-time
    xpT at reversed indices and writes hsT at reversed slots, so hs_b ends
    up in forward-time order for the emission matmuls.
  - Input projection xp is injected into the LSTM gate PSUM via an
    identity-stationary matmul, eliminating the per-step DVE add; tanh
    reads PSUM directly on the Activation engine.
  - The v gate product runs on GpSimd, balancing DVE.
  - P1 bias-adds and P3's exp(feats) run on the Activation engine with
    per-partition bias (bout folded into the exp).
  - No max-shift in the CRF: E = exp(W h + b) directly; periodic renorm
    keeps the exp-domain scan in f32 range.
  - CRF scan in bf16 (single HW matmul per step instead of an fp32 pair),
    split into two staggered 4-row chains with renorm every 16 steps.
"""

import sys

sys.path.insert(0, "/opt/trn_rl_repo")

import numpy as np
import ml_dtypes

import concourse.bass as bass
from concourse import bacc
import concourse.tile as tile
from concourse import mybir
from concourse import bass_isa
from concourse.bass import IndirectOffsetOnAxis
from concourse.bass_utils import run_bass_kernel_spmd
from concourse.masks import make_identity

F32 = mybir.dt.float32
BF16 = mybir.dt.bfloat16
I32 = mybir.dt.int32
ALU = mybir.AluOpType
AF = mybir.ActivationFunctionType
AX = mybir.AxisListType

B, L, E, H, C = 64, 256, 256, 256, 20
G = 4 * H
NCORES = 8
BC = B // NCORES            # batch rows per core
CH = 8                      # gate-hidden chunks of 128 (c = gate*2 + half)
NT = (L * BC) // 128        # token tiles = 16
TPT = 128 // BC             # timesteps per token tile = 16
REN = 16                    # CRF renorm interval (per chain)
NREN = L // REN             # 16
HBC = BC // 2               # rows per CRF chain = 4
START, STOP = 18, 19

_CACHE = {}
DEBUG = False
SPEC_W = 8     # LSTM speculative warmup steps
CSPEC_W = 8    # CRF speculative warmup steps
NH = L // 2     # 128


def _build_module():
    nc = bacc.Bacc(None, target_bir_lowering=False, debug=False)

    # ---- DRAM I/O ----
    d_embed = nc.dram_tensor("embed_bf", [50000, E], BF16, kind="ExternalInput")
    d_idxf = nc.dram_tensor("idx_f", [128, NT], I32, kind="ExternalInput")
    d_wih = nc.dram_tensor("wih", [128, 2, 2, CH, 128], BF16, kind="ExternalInput")
    d_whh = nc.dram_tensor("whh", [128, 2, 2, CH, 128], BF16, kind="ExternalInput")
    d_xbias = nc.dram_tensor("xbias", [128, 2, CH], F32, kind="ExternalInput")
    d_h0 = nc.dram_tensor("h0T", [128, 2, 2, BC], BF16, kind="ExternalInput")
    d_c0 = nc.dram_tensor("c0T", [128, 2, 2, BC], F32, kind="ExternalInput")
    d_wout = nc.dram_tensor("woutT", [128, 2, 2, C], BF16, kind="ExternalInput")
    d_bout = nc.dram_tensor("bout", [C, 1], F32, kind="ExternalInput")
    d_transT = nc.dram_tensor("transT", [C, C], F32, kind="ExternalInput")
    d_transTb = nc.dram_tensor("transTb", [C, C], BF16, kind="ExternalInput")
    d_tstop = nc.dram_tensor("tstop", [C, 1], F32, kind="ExternalInput")
    d_ohprev = nc.dram_tensor("ohprev", [C, BC, L], BF16, kind="ExternalInput")
    d_ohcur = nc.dram_tensor("ohcur", [C, BC, L], F32, kind="ExternalInput")
    d_ohcur_tb = nc.dram_tensor("ohcur_tb", [C, L, BC], F32, kind="ExternalInput")
    d_ohlast = nc.dram_tensor("ohlast", [C, BC], F32, kind="ExternalInput")
    d_a0 = nc.dram_tensor("a0", [C, BC], BF16, kind="ExternalInput")
    d_gold = nc.dram_tensor("gold_out", [1, BC], F32, kind="ExternalOutput")
    d_paf = nc.dram_tensor("paf_out", [1, BC], F32, kind="ExternalOutput")
    d_sall = nc.dram_tensor("sall_out", [1, BC, NREN], F32, kind="ExternalOutput")
    if DEBUG:
        d_dbg_eT = nc.dram_tensor("dbg_eT", [C, L, BC], F32, kind="ExternalOutput")
        d_dbg_gsum = nc.dram_tensor("dbg_gsum", [C, BC], F32, kind="ExternalOutput")
        d_dbg_gold = nc.dram_tensor("dbg_gold", [1, BC], F32, kind="ExternalOutput")
        d_dbg_hf = nc.dram_tensor("dbg_hf", [128, 2, BC], BF16, kind="ExternalOutput")
        d_dbg_hb = nc.dram_tensor("dbg_hb", [128, 2, BC], BF16, kind="ExternalOutput")
        d_dbg_sall = nc.dram_tensor("dbg_sall", [1, BC, NREN], F32, kind="ExternalOutput")
        d_dbg_xp = nc.dram_tensor("dbg_xp", [128, 8, CH, BC], BF16, kind="ExternalOutput")
        d_dbg_cst = nc.dram_tensor("dbg_cst", [128, 2, 2, BC], F32, kind="ExternalOutput")

    with tile.TileContext(nc) as tc:
        with (
            tc.tile_pool(name="persist", bufs=1) as pp,
            tc.tile_pool(name="work", bufs=3) as wp,
            tc.tile_pool(name="lstm", bufs=3) as lp,
        ):
            # ---- persistent SBUF ----
            wih_sb = pp.tile([128, 2, 2, CH, 128], BF16, tag="wih")
            whh_sb = pp.tile([128, 2, 2, CH, 128], BF16, tag="whh")
            xbias_sb = pp.tile([128, 2, CH], F32, tag="xbias")
            wout_sb = pp.tile([128, 2, 2, C], BF16, tag="wout")
            bout_sb = pp.tile([C, 1], F32, tag="bout")
            transT_sb = pp.tile([C, C], F32, tag="transT")
            transTb_sb = pp.tile([C, C], BF16, tag="transTb")
            tstop_sb = pp.tile([C, 1], F32, tag="tstop")
            ohprev_sb = pp.tile([C, BC, L], BF16, tag="ohprev")
            ohcur_sb = pp.tile([C, BC, L], F32, tag="ohcur")
            ohcur_tb_sb = pp.tile([C, L, BC], F32, tag="ohcur_tb")
            ohlast_sb = pp.tile([C, BC], F32, tag="ohlast")
            idxf_sb = pp.tile([128, NT], I32, tag="idxf")
            ident128 = pp.tile([128, 128], BF16, tag="id128")
            xTf = pp.tile([128, 2, NT, 128], BF16, tag="xTf")
            # xp^T: [ghid-part, t, chunk, b]  (both dirs in forward time order)
            xpT = [pp.tile([128, L, CH, BC], BF16, name=f"xpT{d}", tag=f"xpT{d}") for d in (0, 1)]
            # H history: [hid-part, k-half, slot(0..L), b]
            # dir0: init slot 0, step s reads s, writes s+1 (h_f[t] at slot t+1)
            # dir1: init slot L, step s reads L-s, writes L-1-s (h_b[t] at slot t)
            hsT = [pp.tile([128, 2, L + 1, BC], BF16, name=f"hsT{d}", tag=f"hsT{d}") for d in (0, 1)]
            cst = [[pp.tile([128, 2, BC], F32, name=f"cst{d}{hf}", tag=f"cst{d}{hf}")
                    for hf in (0, 1)] for d in (0, 1)]
            hwu = [pp.tile([128, 2, 2, BC], BF16, name=f"hwu{d}", tag=f"hwu{d}") for d in (0, 1)]
            zro = pp.tile([128, 2, BC], BF16, tag="zro")
            eT = pp.tile([C, L, BC], F32, tag="eT")
            pplus = pp.tile([C, C], BF16, tag="pplus")
            wstop = pp.tile([C, 1], BF16, tag="wstop")
            # applied renorm scales (exact f32 bookkeeping): [1, chain*HBC, NREN]
            sall = pp.tile([1, BC, NREN], F32, tag="sall")
            ones1 = pp.tile([1, C], F32, tag="ones1")
            ones20c = pp.tile([C, 1], BF16, tag="ones20c")
            ones20f = pp.tile([C, 1], F32, tag="ones20f")
            avec = [pp.tile([C, 2, BC], BF16, name=f"avec{x}", tag=f"avec{x}") for x in range(4)]
            gsum = pp.tile([C, BC], F32, tag="gsum")
            gold_sb = pp.tile([1, BC], F32, tag="gold")
            cnt_sb = pp.tile([C, BC], F32, tag="cnt")

            # ---- load constants ----
            nc.sync.dma_start(out=wih_sb[:], in_=d_wih[:])
            nc.sync.dma_start(out=whh_sb[:], in_=d_whh[:])
            nc.sync.dma_start(out=xbias_sb[:], in_=d_xbias[:])
            nc.sync.dma_start(out=wout_sb[:], in_=d_wout[:])
            nc.sync.dma_start(out=bout_sb[:], in_=d_bout[:])
            nc.sync.dma_start(out=transT_sb[:], in_=d_transT[:])
            nc.sync.dma_start(out=transTb_sb[:], in_=d_transTb[:])
            nc.sync.dma_start(out=tstop_sb[:], in_=d_tstop[:])
            nc.sync.dma_start(out=ohprev_sb[:], in_=d_ohprev[:])
            nc.sync.dma_start(out=ohcur_sb[:], in_=d_ohcur[:])
            nc.sync.dma_start(out=ohcur_tb_sb[:], in_=d_ohcur_tb[:])
            nc.sync.dma_start(out=ohlast_sb[:], in_=d_ohlast[:])
            nc.sync.dma_start(out=idxf_sb[:], in_=d_idxf[:])
            nc.sync.dma_start(out=hsT[0][:, :, 0, :], in_=d_h0[:, 0, :, :])
            nc.sync.dma_start(out=hsT[1][:, :, L, :], in_=d_h0[:, 1, :, :])
            for d in (0, 1):
                nc.sync.dma_start(out=cst[d][0][:], in_=d_c0[:, d, :, :])
                nc.vector.memset(cst[d][1][:], 0.0)
            nc.vector.memset(zro[:], 0.0)
            make_identity(nc, ident128[:])
            nc.vector.memset(ones1[:], 1.0)
            nc.vector.memset(ones20c[:], 1.0)
            nc.vector.memset(ones20f[:], 1.0)

            # P+ = exp(transT) in bf16;  wstop = exp(T[STOP,:]) in bf16
            nc.scalar.activation(pplus[:], transT_sb[:], AF.Exp)
            nc.scalar.activation(wstop[:], tstop_sb[:], AF.Exp)

            # A0: time-chain 0 starts at onehot(START); chains 1-3 warm up
            # from uniform over CSPEC_W steps, junction-renormalized.
            nc.vector.memset(avec[0][:], 0.0)
            nc.sync.dma_start(out=avec[0][:, 0, :], in_=d_a0[:])
            for x in (1, 2, 3):
                nc.vector.memset(avec[x][:], 1.0)

            # ---- gold transition score (inputs only; runs during P1) ----
            # pu = trans @ ohprev ; gsum = sum_t (pu * ohcur)
            ps_gold = tc.tile_pool(name="ps_gold", bufs=1, space="PSUM")
            psG = ps_gold.__enter__()
            pu = psG.tile([C, BC * L], F32, tag="pu")
            for n in range(4):
                nc.tensor.matmul(
                    pu[:, n * 512 : (n + 1) * 512],
                    transTb_sb[:],
                    ohprev_sb[:].rearrange("p b t -> p (b t)")[
                        :, n * 512 : (n + 1) * 512
                    ],
                    start=True,
                    stop=True,
                )
            prod = pp.tile([C, BC, L], F32, tag="prod")
            nc.vector.scalar_tensor_tensor(
                out=prod[:].rearrange("p b t -> p (b t)"), in0=pu[:], scalar=0.0,
                in1=ohcur_sb[:].rearrange("p b t -> p (b t)"), op0=ALU.add, op1=ALU.mult,
            )
            nc.vector.tensor_reduce(out=gsum[:], in_=prod[:], axis=AX.X, op=ALU.add)
            # cnt[c,b] = #timesteps with tag c (for the bout term of emissions)
            nc.vector.tensor_reduce(out=cnt_sb[:], in_=ohcur_sb[:], axis=AX.X, op=ALU.add)
            # cnt *= bout (per-tag emission bias counts); gsum += cnt
            nc.gpsimd.tensor_tensor(
                out=cnt_sb[:], in0=cnt_sb[:],
                in1=bout_sb[:].to_broadcast([C, BC]), op=ALU.mult,
            )
            nc.gpsimd.tensor_add(gsum[:], gsum[:], cnt_sb[:])
            ps_gold.__exit__(None, None, None)

            # ---- P1: gather + transpose + input projection ----
            # Projection groups for (d0,h0) are emitted right after tile 0-7
            # transposes so they execute while gathers 8-15 are still running.
            ps_p1 = tc.tile_pool(name="ps_p1", bufs=2, space="PSUM")
            psA = ps_p1.__enter__()

            def gather_transpose(ti):
                gx = wp.tile([128, E], BF16, tag="gx")
                nc.gpsimd.indirect_dma_start(
                    out=gx[:],
                    out_offset=None,
                    in_=d_embed[:],
                    in_offset=IndirectOffsetOnAxis(ap=idxf_sb[:, ti : ti + 1], axis=0),
                )
                for k in (0, 1):
                    pt = psA.tile([128, 128], BF16, tag="ptr")
                    nc.tensor.transpose(
                        pt[:], gx[:, k * 128 : (k + 1) * 128], ident128[:]
                    )
                    nc.vector.tensor_copy(xTf[:, k, ti, :], pt[:])

            def proj(d, h):
                for c in range(CH):
                    pj = psA.tile([128, 1024], F32, tag="pj", bufs=3)
                    for k in (0, 1):
                        for nb in (0, 1):
                            nc.tensor.matmul(
                                pj[:, nb * 512 : (nb + 1) * 512],
                                wih_sb[:, d, k, c, :],
                                xTf[:, k, h * 8 + nb * 4 : h * 8 + (nb + 1) * 4, :],
                                start=(k == 0),
                                stop=(k == 1),
                            )
                    # xp^T[t, c, b] = pj + bias (split across ACT and DVE)
                    if (c + h) % 2 == 0:
                        nc.scalar.activation(
                            out=xpT[d][:, h * 128 : (h + 1) * 128, c, :],
                            in_=pj[:].rearrange("p (tt b) -> p tt b", b=BC),
                            func=AF.Identity,
                            bias=xbias_sb[:, d, c : c + 1],
                        )
                    else:
                        nc.vector.tensor_scalar(
                            out=xpT[d][:, h * 128 : (h + 1) * 128, c, :],
                            in0=pj[:].rearrange("p (tt b) -> p tt b", b=BC),
                            scalar1=xbias_sb[:, d, c : c + 1],
                            scalar2=None,
                            op0=ALU.add,
                        )

            for ti in range(8):
                gather_transpose(ti)
            proj(0, 0)
            for ti in range(8, NT):
                gather_transpose(ti)
            proj(1, 1)
            proj(1, 0)
            proj(0, 1)
            ps_p1.__exit__(None, None, None)

            # ---- P2: LSTM recurrence, 4 speculative chains ----
            # (dir, half): half 0 covers steps 0..NH-1 exactly; half 1 warms up
            # from zero state over steps NH-SPEC_W..NH-1 (scratch ping-pong),
            # then runs steps NH..L-1 writing the real hsT slots.
            ps_p2 = tc.tile_pool(name="ps_p2", bufs=6, space="PSUM")
            psB = ps_p2.__enter__()

            def chain_slot(d, hf, j):
                # -> (read_view, write_view, xp_time_index) or None
                if hf == 0:
                    if j >= NH:
                        return None
                    s = j
                    if d == 0:
                        rd, wr = hsT[0][:, :, s, :], hsT[0][:, :, s + 1, :]
                    else:
                        rd, wr = hsT[1][:, :, L - s, :], hsT[1][:, :, L - 1 - s, :]
                else:
                    if j >= NH + SPEC_W:
                        return None
                    if j < SPEC_W:
                        s = NH - SPEC_W + j
                        rd = zro[:] if j == 0 else hwu[d][:, :, (j + 1) % 2, :]
                        wr = hwu[d][:, :, j % 2, :]
                    else:
                        s = NH + (j - SPEC_W)
                        if j == SPEC_W:
                            rd = hwu[d][:, :, (SPEC_W - 1) % 2, :]
                        else:
                            rd = (hsT[0][:, :, s, :] if d == 0
                                  else hsT[1][:, :, L - s, :])
                        wr = (hsT[0][:, :, s + 1, :] if d == 0
                              else hsT[1][:, :, L - 1 - s, :])
                xi = s if d == 0 else L - 1 - s
                return rd, wr, xi

            for j in range(NH + SPEC_W):
                for d, hf in ((0, 0), (1, 0), (0, 1), (1, 1)):
                    cs = chain_slot(d, hf, j)
                    if cs is None:
                        continue
                    rd, wr, xi = cs
                    cd = cst[d][hf]
                    pg = psB.tile([128, CH, BC], F32, tag="pg")
                    nc.tensor.matmul(
                        pg[:].rearrange("p c b -> p (c b)"),
                        ident128[:],
                        xpT[d][:, xi, :, :].rearrange("p c b -> p (c b)"),
                        start=True,
                        stop=False,
                        skip_group_check=True,
                    )
                    for c in range(CH):
                        for k in (0, 1):
                            nc.tensor.matmul(
                                pg[:, c, :],
                                whh_sb[:, d, k, c, :],
                                rd[:, k, :],
                                start=False,
                                stop=(c == CH - 1 and k == 1),
                                skip_group_check=True,
                            )
                    th = lp.tile([128, CH, BC], F32, tag="th")
                    nc.scalar.activation(th[:], pg[:], AF.Tanh)
                    v = lp.tile([128, 2, BC], F32, tag="v")
                    nc.vector.scalar_tensor_tensor(
                        out=v[:], in0=th[:, 2:4, :], scalar=1.0, in1=cd[:],
                        op0=ALU.add, op1=ALU.mult,
                    )
                    u = lp.tile([128, 2, BC], F32, tag="u")
                    nc.vector.scalar_tensor_tensor(
                        out=u[:], in0=th[:, 0:2, :], scalar=1.0, in1=th[:, 4:6, :],
                        op0=ALU.add, op1=ALU.mult,
                    )
                    nc.vector.scalar_tensor_tensor(
                        out=cd[:], in0=v[:], scalar=0.5, in1=u[:],
                        op0=ALU.mult, op1=ALU.add,
                    )
                    tcc = lp.tile([128, 2, BC], F32, tag="tcc")
                    nc.scalar.activation(tcc[:], cd[:], AF.Tanh, scale=0.5)
                    nc.vector.scalar_tensor_tensor(
                        out=wr, in0=th[:, 6:8, :], scalar=1.0,
                        in1=tcc[:], op0=ALU.add, op1=ALU.mult,
                    )
            ps_p2.__exit__(None, None, None)

            # ---- P3: emissions E = exp(sum_d Wout_d @ H_d + bout) ----
            ps_p3 = tc.tile_pool(name="ps_p3", bufs=1, space="PSUM")
            psC = ps_p3.__enter__()
            pf = psC.tile([C, L * BC], F32, tag="pf")
            for d in (0, 1):
                for k in (0, 1):
                    for n in range(4):
                        base = 1 + n * 64 if d == 0 else n * 64
                        nc.tensor.matmul(
                            pf[:, n * 512 : (n + 1) * 512],
                            wout_sb[:, d, k, :],
                            hsT[d][:, k, base : base + 64, :],
                            start=(d == 0 and k == 0),
                            stop=(d == 1 and k == 1),
                        )
            nc.scalar.activation(
                out=eT[:].rearrange("p t b -> p (t b)"),
                in_=pf[:],
                func=AF.Exp,
                bias=bout_sb[:, 0:1],
            )
            # gold emissions: sum_t pf[gold tag] (bout term already via cnt)
            prod2 = pp.tile([C, L, BC], F32, tag="prod2")
            nc.vector.scalar_tensor_tensor(
                out=prod2[:].rearrange("p t b -> p (t b)"), in0=pf[:], scalar=0.0,
                in1=ohcur_tb_sb[:].rearrange("p t b -> p (t b)"), op0=ALU.add, op1=ALU.mult,
            )
            gsum2 = pp.tile([C, BC], F32, tag="gsum2")
            nc.vector.tensor_reduce(
                out=gsum2[:],
                in_=prod2[:].rearrange("p t b -> p b t"),
                axis=AX.X, op=ALU.add,
            )
            nc.gpsimd.tensor_add(gsum[:], gsum[:], gsum2[:])
            # + T[STOP, tag_last]
            stopterm = pp.tile([C, BC], F32, tag="stopterm")
            nc.gpsimd.tensor_tensor(
                out=stopterm[:], in0=ohlast_sb[:],
                in1=tstop_sb[:].to_broadcast([C, BC]), op=ALU.mult,
            )
            nc.gpsimd.tensor_add(gsum[:], gsum[:], stopterm[:])
            ps_p3.__exit__(None, None, None)

            # ---- P5: CRF forward scan, two staggered 4-row chains ----
            ps_p5 = tc.tile_pool(name="ps_p5", bufs=2, space="PSUM")
            psD = ps_p5.__enter__()
            # gold reduce over tags (PE while idle-ish): gold = ones20c^T @ gsum
            pgold = psD.tile([1, BC], F32, tag="pgold", bufs=1)
            nc.tensor.matmul(pgold[:], ones20f[:], gsum[:], start=True, stop=True)
            nc.vector.tensor_copy(gold_sb[:], pgold[:])

            NQ = L // 4
            for j in range(NQ + CSPEC_W):
                for x in range(4):
                    if x == 0:
                        if j >= NQ:
                            continue
                        t = j
                        warm_end = False
                        log_rn = (t % REN == REN - 1)
                    else:
                        if j < CSPEC_W:
                            t = x * NQ - CSPEC_W + j
                            warm_end = (j == CSPEC_W - 1)
                            log_rn = False
                        else:
                            t = x * NQ + (j - CSPEC_W)
                            warm_end = False
                            log_rn = (t % REN == REN - 1)
                    cur = (j + 1) % 2
                    pa = psD.tile([C, BC], F32, tag="pa", bufs=6)
                    nc.tensor.matmul(
                        pa[:], pplus[:], avec[x][:, j % 2, :], start=True, stop=True
                    )
                    nc.vector.scalar_tensor_tensor(
                        out=avec[x][:, cur, :], in0=pa[:], scalar=0.0,
                        in1=eT[:, t, :], op0=ALU.add, op1=ALU.mult,
                    )
                    if log_rn or warm_end:
                        ssum = wp.tile([C, BC], F32, tag=f"ssum{x}")
                        nc.gpsimd.partition_all_reduce(
                            ssum[:], avec[x][:, cur, :], channels=C,
                            reduce_op=bass_isa.ReduceOp.add,
                        )
                        srec = wp.tile([C, BC], F32, tag=f"srec{x}")
                        nc.vector.reciprocal(srec[:], ssum[:])
                        if log_rn:
                            nc.vector.tensor_copy(sall[0:1, :, t // REN], srec[0:1, :])
                        nc.vector.scalar_tensor_tensor(
                            out=avec[x][:, cur, :],
                            in0=avec[x][:, cur, :], scalar=0.0,
                            in1=srec[:], op0=ALU.add, op1=ALU.mult,
                        )

            # ---- P6: ship gold/paf/sall to host (lns done in float64 there;
            # the ACT Ln table is invalid for the ~1e-24 srec magnitudes) ----
            paf = psD.tile([1, BC], F32, tag="paf", bufs=1)
            nc.tensor.matmul(
                paf[:], wstop[:], avec[3][:, (NQ + CSPEC_W) % 2, :],
                start=True, stop=True,
            )
            paf_sb = wp.tile([1, BC], F32, tag="paf_sb")
            nc.vector.tensor_copy(paf_sb[:], paf[:])
            nc.sync.dma_start(out=d_gold[:], in_=gold_sb[:])
            nc.sync.dma_start(out=d_paf[:], in_=paf_sb[:])
            nc.sync.dma_start(out=d_sall[:], in_=sall[:])
            if DEBUG:
                nc.sync.dma_start(out=d_dbg_eT[:], in_=eT[:])
                nc.sync.dma_start(out=d_dbg_gsum[:], in_=gsum[:])
                nc.sync.dma_start(out=d_dbg_gold[:], in_=gold_sb[:])
                nc.sync.dma_start(out=d_dbg_hf[:], in_=hsT[0][:, :, L, :])
                nc.sync.dma_start(out=d_dbg_hb[:], in_=hsT[1][:, :, 0, :])
                nc.sync.dma_start(out=d_dbg_sall[:], in_=sall[:])
                nc.sync.dma_start(out=d_dbg_xp[:], in_=xpT[0][:, 0:8, :, :])
                for d in (0, 1):
                    nc.sync.dma_start(out=d_dbg_cst[:, d, :, :], in_=cst[d][:])
            ps_p5.__exit__(None, None, None)

    nc.finalize()
    return nc


def _prep_inmaps(inputs):
    bf = ml_dtypes.bfloat16
    sent = np.asarray(inputs["sentences"])
    tags = np.asarray(inputs["tags"])
    embed = np.asarray(inputs["embed"], dtype=np.float32)
    trans = np.asarray(inputs["transitions"], dtype=np.float32)
    h0 = np.asarray(inputs["h0"], dtype=np.float32)
    c0 = np.asarray(inputs["c0"], dtype=np.float32)
    W_out = np.asarray(inputs["W_out"], dtype=np.float32)
    b_out = np.asarray(inputs["b_out"], dtype=np.float32)

    rs = np.full((G, 1), 0.5, np.float32)
    rs[2 * H : 3 * H] = 1.0  # g-gate rows unscaled

    embed_bf = np.ascontiguousarray(embed.astype(bf))

    def chunk_weights(W):  # W [G, K_in] -> [128, 2, CH, 128] = [p, k, c, m]
        Kin = W.shape[1]
        Wr = W.reshape(4, 2, 128, Kin // 128, 128)  # [gate, hh, m, k, p]
        return np.ascontiguousarray(Wr.transpose(4, 3, 0, 1, 2).reshape(128, Kin // 128, CH, 128))

    wih = np.zeros((128, 2, 2, CH, 128), np.float32)
    whh = np.zeros((128, 2, 2, CH, 128), np.float32)
    xbias = np.zeros((128, 2, CH), np.float32)
    for d, (Wih, Whh, b) in enumerate(
        [
            (inputs["Wih_f"], inputs["Whh_f"], inputs["b_f"]),
            (inputs["Wih_b"], inputs["Whh_b"], inputs["b_b"]),
        ]
    ):
        Wih = np.asarray(Wih, np.float32) * rs
        Whh = np.asarray(Whh, np.float32) * rs * 0.5
        bt = np.asarray(b, np.float32) * rs[:, 0]
        wih[:, d] = chunk_weights(Wih)
        whh[:, d] = chunk_weights(Whh)
        xbias[:, d] = bt.reshape(4, 2, 128).transpose(2, 0, 1).reshape(128, CH)
    wih = np.ascontiguousarray(wih.astype(bf))
    whh = np.ascontiguousarray(whh.astype(bf))

    # wout^T [p, d, k, m] = 0.5 * W_out[m, d*256 + k*128 + p]
    wout = np.ascontiguousarray(
        (0.5 * W_out).reshape(C, 2, 2, 128).transpose(3, 1, 2, 0).astype(bf)
    )
    bout = np.ascontiguousarray(b_out[:, None])
    transT = np.ascontiguousarray(trans.T)
    transTb = np.ascontiguousarray(trans.T.astype(bf))
    tstop = np.ascontiguousarray(trans[STOP, :][:, None])

    in_maps = []
    for q in range(NCORES):
        bs = slice(q * BC, (q + 1) * BC)
        sq = sent[bs]  # [BC, L]
        tq = tags[bs]
        idx_f = np.ascontiguousarray(
            sq.T.reshape(NT, TPT, BC).transpose(1, 2, 0).reshape(128, NT).astype(np.int32)
        )
        h0q = np.ascontiguousarray(
            (2.0 * h0[:, bs, :]).reshape(2, BC, 2, 128).transpose(3, 0, 2, 1).astype(bf)
        )
        c0q = np.ascontiguousarray(
            (2.0 * c0[:, bs, :]).reshape(2, BC, 2, 128).transpose(3, 0, 2, 1).astype(np.float32)
        )
        te_prev = np.concatenate(
            [np.full((BC, 1), START, tags.dtype), tq[:, :-1]], axis=1
        )  # prev tag at each t
        ar = np.arange(C)
        ohprev = (ar[:, None, None] == te_prev[None, :, :]).astype(np.float32)
        ohcur = (ar[:, None, None] == tq[None, :, :]).astype(np.float32)
        ohcur_tb = np.ascontiguousarray(ohcur.transpose(0, 2, 1))
        ohlast = (ar[:, None] == tq[None, :, L - 1]).astype(np.float32)
        a0 = ((ar[:, None] == START) * np.ones((1, BC))).astype(bf)
        in_maps.append(
            {
                "embed_bf": embed_bf,
                "idx_f": idx_f,
                "wih": wih,
                "whh": whh,
                "xbias": xbias,
                "h0T": h0q,
                "c0T": c0q,
                "woutT": wout,
                "bout": bout,
                "transT": transT,
                "transTb": transTb,
                "tstop": tstop,
                "ohprev": np.ascontiguousarray(ohprev.astype(bf)),
                "ohcur": np.ascontiguousarray(ohcur),
                "ohcur_tb": ohcur_tb,
                "ohlast": np.ascontiguousarray(ohlast),
                "a0": np.ascontiguousarray(a0),
            }
        )
    return in_maps


def get_module():
    if "nc" not in _CACHE:
        _CACHE["nc"] = _build_module()
    return _CACHE["nc"]


def _finalize(outs):
    """Host-side: partial = sum_b [ln(paf_b) - sum_r ln(srec_br) - gold_b]."""
    paf = np.asarray(outs["paf_out"], np.float64)[0]
    sall = np.asarray(outs["sall_out"], np.float64)[0]
    gold = np.asarray(outs["gold_out"], np.float64)[0]
    F = np.log(paf) - np.log(sall).sum(axis=1)
    return float((F - gold).sum())


def kernel(**inputs):
    nc = get_module()
    in_maps = _prep_inmaps(inputs)
    res = run_bass_kernel_spmd(nc, in_maps, core_ids=list(range(NCORES)))
    total = sum(_finalize(r) for r in res.results)
    return np.float32(total / B)
